# revision 1
# baseline (speedup 1.0000x reference)
"""Trainium2 Bass kernel for nn_AdaptiveChebBlock (8 NeuronCores).

Sharding: batch b = core//2 (4 batches), row-half j = core%2 (2048 rows each).
Each core computes its 2048 rows of the dynamic top-k adjacency + Chebyshev
propagation for its batch; pair collectives (AllGather over [2c,2c+1]) exchange
the degree vector and T1.

v2 design (vs. full-row DVE top-k baseline):
- fp16 feature path: xnT / w1 / A-build matmuls all fp16 (1 cyc/col on PE).
- rotated node space: after the feature pass, two SBUF->SBUF DMAs (the only
  ops with runtime pid-dependent offsets) reorder columns so this core's own
  2048 rows come first; all compute APs stay static.
- pass 1 top-k: per-512-chunk max8 read directly from PSUM -> 64 candidates
  per row; 4 max8/match_replace rounds on the 64-wide candidate array give
  top-32 values (degree) + the 32nd value (threshold, clamped to >= 0).
  No ReLU pass, no row-side A materialization in SBUF at all.
- masked transposed adjacency MT[col, row]: rebuilt by a second set of fp16
  matmuls (A is symmetric pre-mask), ACT-copied PSUM->MT fp16, then masked
  in place with two fp16 2x-mode DVE tensor_tensor ops against a partition-
  broadcast threshold tensor THRb. (Pool/ACT cannot run TensorTensor on HW.)
  Thr clamp at 0 makes raw-A masking == relu-A masking.
- output stage is transpose-free: h^T/T1^T are transposed once during the
  exchange window, T2^T is assembled per strip from the raw [h,row] psum
  via two TTs against broadcast per-row coefficient vectors.
- pass 3 starts with this core's own half of G2 (from local T1loc), hiding
  the T1 pair-exchange latency under the first half of the T2 matmuls.
"""
import os, sys
os.environ.setdefault("JAX_PLATFORMS", "")
for _p in ("/root/.axon_site/_ro/trn_rl_repo", "/opt/trn_rl_repo"):
    if os.path.isdir(_p):
        if _p not in sys.path:
            sys.path.insert(0, _p)
        break  # use exactly one copy — mixing versions breaks imports

import numpy as np

import concourse.bass as bass
import concourse.bacc as bacc
import concourse.tile as tile
import concourse.mybir as mybir
import concourse.masks as masks
from concourse.bass_utils import run_bass_kernel_spmd

F32 = mybir.dt.float32
F16 = mybir.dt.float16
Alu = mybir.AluOpType
Act = mybir.ActivationFunctionType

KCHEB = 3
TOPK = 32
TELEPORT = 0.1
LN_EPS = 1e-5

# problem shape (hardcoded per spec)
BSZ, NFULL, DDIM = 4, 4096, 128
HDIM, ODIM = 128, 128
N_CORES = 8

NEG_FILL = -1.0e30


class Cfg:
    def __init__(self, n_nodes, n_rows, use_cc, scalars, flags, gelu=True,
                 chunk=512):
        self.n = n_nodes            # nodes this core sees (columns of A)
        self.r = n_rows             # rows this core owns
        self.NT = n_nodes // 128    # node tiles
        self.RT = n_rows // 128     # row tiles
        self.use_cc = use_cc        # emit pair collectives (8-core mode)
        self.c1, self.c2, self.tg = scalars
        # flags: which optional affine params are non-trivial
        self.lng, self.lnb, self.b1, self.b2 = flags
        self.gelu = gelu            # False only for CoreSim (no Gelu in interp)
        self.chunk = chunk          # top-k candidate chunk width


def _emit(nc, tc, cfg):
    """Emit the whole per-core graph inside TileContext tc."""
    n, r, NT, RT = cfg.n, cfg.r, cfg.NT, cfg.RT
    c1, c2, tg = cfg.c1, cfg.c2, cfg.tg
    gelu_f = Act.Gelu if cfg.gelu else Act.Identity

    # ---- DRAM I/O -------------------------------------------------------
    xf = nc.dram_tensor("xf", [n, DDIM], F32, kind="ExternalInput")       # full batch slice
    xm = nc.dram_tensor("xm", [r, DDIM], F32, kind="ExternalInput")       # my rows
    w1e = nc.dram_tensor("w1e", [DDIM, HDIM], F32, kind="ExternalInput")
    w2e = nc.dram_tensor("w2e", [KCHEB * HDIM, ODIM], F32, kind="ExternalInput")
    lng_e = nc.dram_tensor("lng", [DDIM], F32, kind="ExternalInput")
    lnb_e = nc.dram_tensor("lnb", [DDIM], F32, kind="ExternalInput")
    b1_e = nc.dram_tensor("b1e", [HDIM], F32, kind="ExternalInput")
    b2_e = nc.dram_tensor("b2e", [ODIM], F32, kind="ExternalInput")
    out_e = nc.dram_tensor("out", [r, ODIM], F32, kind="ExternalOutput")

    # DRAM scratch
    h16_dram = nc.dram_tensor("h16_scr", [128, n], F16)   # h fp16 staging for rotation
    thr_dram = nc.dram_tensor("thr_scr", [r], F16)
    q2_dram = nc.dram_tensor("q2_scr", [r], F16)
    cdm2_dram = nc.dram_tensor("cdm2_scr", [r], F16)
    xnt_dram = nc.dram_tensor("xnt_scr", [128, n], F16)   # xn^T staging for rotation
    dm_in = nc.dram_tensor("dm_in", [r], F32)
    t1_in = nc.dram_tensor("t1_in", [r, HDIM], F16)
    # NOTE: Shared addr_space is rejected for 2-rank groups; plain DRAM works.
    dm_out = nc.dram_tensor("dm_out", [n], F32)
    t1_out = nc.dram_tensor("t1_out", [n, HDIM], F16)
    groups = [[0, 1], [2, 3], [4, 5], [6, 7]]

    import contextlib
    stack = contextlib.ExitStack()
    const = stack.enter_context(tc.tile_pool(name="const", bufs=1))
    persist = stack.enter_context(tc.tile_pool(name="persist", bufs=1))

    id16 = const.tile([128, 128], F16, tag="id16")
    masks.make_identity(nc, id16[:])
    w1s16 = const.tile([DDIM, HDIM], F16, tag="w1s16")
    if cfg.lng:
        LNG = const.tile([128, DDIM], F32, tag="LNG")
        nc.sync.dma_start(LNG[:], lng_e.ap().partition_broadcast(128))
    if cfg.lnb:
        LNB = const.tile([128, DDIM], F32, tag="LNB")
        nc.sync.dma_start(LNB[:], lnb_e.ap().partition_broadcast(128))
    if cfg.b1:
        B1R = const.tile([128, HDIM], F32, tag="B1R")
        nc.sync.dma_start(B1R[:], b1_e.ap().partition_broadcast(128))

    MT = persist.tile([128, NT, r], F16, tag="MT")        # adjacency^T, masked in place
    w2s = persist.tile([128, KCHEB, ODIM], F16, tag="w2s")
    hROT = persist.tile([128, n], F16, tag="hROT")        # h fp16, rotated node order
    degM = persist.tile([128, RT], F32, tag="degM")
    dmv = persist.tile([128, 6, RT], F32, tag="dmv")      # [deg|dm12|cdm1|q1|q2|cdm2]
    dm12rot = persist.tile([128, NT], F32, tag="dm12rot")  # dm12, rotated node order
    thrM = persist.tile([128, RT], F16, tag="thrM")       # per-row-tile thresholds
    THRb = persist.tile([128, r], F16, tag="THRb")        # thresholds bcast over partitions
    m01a = persist.tile([128, r], F16, tag="m01a")        # mask bits scratch (DVE)
    m01p = persist.tile([128, r], F16, tag="m01p")        # mask bits scratch (Pool)

    # my row-window start (in nodes): core parity picks the half.
    # pid is a RUNTIME scalar: noff/ooff may only appear in DMA source offsets.
    pid = nc.partition_id()
    noff = (pid % 2) * (n - r)        # my half start
    ooff = ((pid + 1) % 2) * (n - r)  # other half start

    # =====================================================================
    # Feature pass: x tiles -> LN -> h=gelu(.@w1) -> h16, xn -> xnT16
    # =====================================================================
    early_stack = contextlib.ExitStack()
    early = early_stack.enter_context(tc.tile_pool(name="early", bufs=1))
    xnROT = early.tile([128, n], F16, tag="xnROT")     # my half first

    def _moments_arith(pool, nt, bnst, tagp, eps):
        """mean + 1/sqrt(var+eps) (+ sum-of-squares) from bn_stats output."""
        me, mo = bnst[:, :, 1], bnst[:, :, 4]
        m2e, m2o = bnst[:, :, 2], bnst[:, :, 5]
        mu = pool.tile([128, nt], F32, tag=tagp + "_mu", name="mu")
        rstd = pool.tile([128, nt], F32, tag=tagp + "_rstd", name="rstd")
        ssq = pool.tile([128, nt], F32, tag=tagp + "_ssq", name="ssq")
        dl = pool.tile([128, nt], F32, tag=tagp + "_dl", name="dl")
        nc.vector.tensor_tensor(dl[:], me, mo, Alu.subtract)
        nc.vector.tensor_tensor(dl[:], dl[:], dl[:], Alu.mult)       # delta^2
        nc.vector.tensor_tensor(mu[:], me, mo, Alu.add)
        nc.vector.tensor_scalar_mul(mu[:], mu[:], 0.5)               # mean
        nc.vector.tensor_tensor(rstd[:], m2e, m2o, Alu.add)
        nc.vector.scalar_tensor_tensor(rstd[:], dl[:], float(DDIM) / 4.0, rstd[:],
                                       op0=Alu.mult, op1=Alu.add)    # M2 total
        nc.vector.tensor_tensor(ssq[:], mu[:], mu[:], Alu.mult)
        nc.vector.scalar_tensor_tensor(ssq[:], ssq[:], float(DDIM), rstd[:],
                                       op0=Alu.mult, op1=Alu.add)    # sum sq
        nc.vector.tensor_scalar(rstd[:], rstd[:], 1.0 / DDIM, eps,
                                op0=Alu.mult, op1=Alu.add)           # var + eps
        nc.scalar.activation(rstd[:], rstd[:], Act.Sqrt)
        nc.vector.reciprocal(rstd[:], rstd[:])
        return mu, rstd, ssq

    def feature_pass(src, nt, pool, tpool, psum, psum2):
        xall = pool.tile([128, nt, DDIM], F32, tag="ff_xall")
        hall = pool.tile([128, nt, HDIM], F32, tag="ff_hall")
        bnx = pool.tile([128, nt, 6], F32, tag="ff_bnx")
        bnh = pool.tile([128, nt, 6], F32, tag="ff_bnh")
        for g0 in range(0, nt, 4):
            nc.sync.dma_start(
                xall[:, g0:g0 + 4, :],
                src.ap().rearrange("(g p) d -> p g d", p=128)[:, g0:g0 + 4, :])
        w1f = pool.tile([DDIM, HDIM], F32, tag="ff_w1f")
        nc.sync.dma_start(w1f[:], w1e[:])
        nc.scalar.copy(w1s16[:], w1f[:])
        # processed in quarter-blocks so each block's matmul/gelu/norm chain
        # overlaps the next block's stats instead of waiting on them
        nh = nt // 4
        for hf in range(4):
            lo = hf * nh
            for g in range(lo, lo + nh):
                nc.vector.bn_stats(bnx[:, g, :], xall[:, g, :])
            mu, rstd, _ = _moments_arith(pool, nh, bnx[:, lo:lo + nh, :],
                                         f"ffx{hf}", LN_EPS)
            for g0 in range(lo, lo + nh, 4):
                ph4 = psum2.tile([128, 4, 128], F32, tag="fp_ph4")
                ptx4 = psum.tile([128, 4, 128], F16, tag="fp_ptx4")
                xlnT4 = tpool.tile([128, 4, DDIM], F16, tag="fp_xlnT4")
                for gi in range(4):
                    g = g0 + gi
                    xln = tpool.tile([128, DDIM], F16, tag="fp_xln")
                    nc.vector.tensor_scalar(xln[:], xall[:, g, :],
                                            mu[:, g - lo:g - lo + 1],
                                            rstd[:, g - lo:g - lo + 1],
                                            op0=Alu.subtract, op1=Alu.mult)
                    if cfg.lng:
                        nc.vector.tensor_tensor(xln[:], xln[:], LNG[:], Alu.mult)
                    if cfg.lnb:
                        nc.vector.tensor_tensor(xln[:], xln[:], LNB[:], Alu.add)
                    nc.tensor.transpose(ptx4[:, gi, :], xln[:], id16[:])
                nc.scalar.copy(xlnT4[:], ptx4[:])
                for gi in range(4):
                    nc.tensor.matmul(ph4[:, gi, :], xlnT4[:, gi, :], w1s16[:])
                    if cfg.b1:
                        nc.vector.tensor_tensor(ph4[:, gi, :], ph4[:, gi, :],
                                                B1R[:], Alu.add)
                nc.scalar.activation(hall[:, g0:g0 + 4, :], ph4[:], gelu_f)
                h16s = tpool.tile([128, 4, HDIM], F16, tag="fp_h16s")
                nc.vector.tensor_copy(h16s[:], hall[:, g0:g0 + 4, :])
                nc.sync.dma_start(
                    h16_dram.ap().rearrange("p (g d) -> p g d", d=HDIM)[:, g0:g0 + 4, :],
                    h16s[:])
                for gi in range(4):
                    g = g0 + gi
                    nc.vector.bn_stats(bnh[:, g, :], hall[:, g, :])
            _, _, ssqh = _moments_arith(pool, nh, bnh[:, lo:lo + nh, :],
                                        f"ffh{hf}", 0.0)
            invh = pool.tile([128, nh], F32, tag=f"ff_invh{hf}", name="invh")
            nc.scalar.activation(invh[:], ssqh[:], Act.Sqrt)
            nc.vector.tensor_scalar_max(invh[:], invh[:], 1e-12)
            nc.vector.reciprocal(invh[:], invh[:])
            for g0 in range(lo, lo + nh, 4):
                xn4 = tpool.tile([128, 4, HDIM], F16, tag="fp_xn4")
                pt4 = psum.tile([128, 4, 128], F16, tag="fp_pt4")
                for gi in range(4):
                    g = g0 + gi
                    nc.vector.tensor_scalar_mul(xn4[:, gi, :], hall[:, g, :],
                                                invh[:, g - lo:g - lo + 1])
                    nc.tensor.transpose(pt4[:, gi, :], xn4[:, gi, :], id16[:])
                xns = tpool.tile([128, 4, 128], F16, tag="fp_xns")
                nc.scalar.copy(xns[:], pt4[:])
                nc.sync.dma_start(
                    xnt_dram.ap().rearrange("p (g d) -> p g d", d=128)[:, g0:g0 + 4, :],
                    xns[:])

    with tc.tile_pool(name="p0", bufs=1) as p0w, \
         tc.tile_pool(name="p0t", bufs=3) as p0t, \
         tc.tile_pool(name="p0ps", bufs=3, space="PSUM") as p0ps, \
         tc.tile_pool(name="p0ps2", bufs=2, space="PSUM") as p0ps2:
        feature_pass(xf, NT, p0w, p0t, p0ps, p0ps2)
        for k in range(KCHEB):
            w2f = p0w.tile([128, ODIM], F32, tag="w2f")
            nc.sync.dma_start(w2f[:], w2e[k * 128:(k + 1) * 128, :])
            nc.scalar.copy(w2s[:, k, :], w2f[:])

    # Rotate into "my half first" node order (runtime offsets, DMA only).
    nc.sync.dma_start(xnROT[:, 0:r], xnt_dram.ap()[:, bass.ds(noff, r)])
    nc.scalar.dma_start(xnROT[:, r:n], xnt_dram.ap()[:, bass.ds(ooff, r)])
    nc.gpsimd.dma_start(hROT[:, 0:r], h16_dram.ap()[:, bass.ds(noff, r)])
    nc.gpsimd.dma_start(hROT[:, r:n], h16_dram.ap()[:, bass.ds(ooff, r)])

    # =====================================================================
    # Pass 1: row-side candidate top-k (DVE from PSUM) + AT rebuild -> MT
    # =====================================================================
    CH = cfg.chunk
    NCH = n // CH                 # candidate chunks per row (8 for 512)
    NQ = n // 512                 # 512-wide matmul strips per row tile
    DEG, DM, CDM1, Q1, Q2, CDM2 = range(6)

    def emit_dm_chain():
        """deg -> dm12 vectors, pair exchange, and threshold broadcast."""
        nc.vector.tensor_scalar(dmv[:, DEG, :], degM[:], c1, c2,
                                op0=Alu.mult, op1=Alu.add)
        nc.scalar.activation(dmv[:, DM, :], dmv[:, DEG, :], Act.Sqrt)
        nc.vector.reciprocal(dmv[:, DM, :], dmv[:, DM, :])
        nc.vector.tensor_scalar_mul(dmv[:, CDM1, :], dmv[:, DM, :], c1)
        nc.vector.tensor_tensor(dmv[:, Q1, :], dmv[:, DM, :], dmv[:, DM, :], Alu.mult)
        nc.vector.tensor_scalar_mul(dmv[:, Q2, :], dmv[:, Q1, :], 2.0 * c2)
        nc.vector.tensor_scalar_mul(dmv[:, Q1, :], dmv[:, Q1, :], c2)
        nc.vector.tensor_scalar_mul(dmv[:, CDM2, :], dmv[:, DM, :], 2.0 * c1)
        nc.sync.dma_start(dm_in.ap().rearrange("(t p) -> p t", p=128), dmv[:, DM, :])
        if cfg.use_cc:
            nc.gpsimd.collective_compute("AllGather", Alu.bypass,
                                         replica_groups=groups,
                                         ins=[dm_in[:].opt()], outs=[dm_out[:].opt()])
        else:
            nc.sync.dma_start(dm_out[0:r], dm_in[:])
            if n > r:
                nc.sync.dma_start(dm_out[r:n], dm_in[:])
        # dm12 in rotated node order: my half first
        nc.sync.dma_start(dm12rot[:, 0:RT],
                          dm_out.ap()[bass.ds(noff, r)].rearrange("(g p) -> p g", p=128))
        nc.sync.dma_start(dm12rot[:, RT:NT],
                          dm_out.ap()[bass.ds(ooff, r)].rearrange("(g p) -> p g", p=128))

    with tc.tile_pool(name="p1c", bufs=2) as p1c, \
         tc.tile_pool(name="p1psA", bufs=3, space="PSUM") as psA, \
         tc.tile_pool(name="p1psT", bufs=2, space="PSUM") as psAT:
        for t in range(RT):
            # --- row-side A strips + per-chunk max8 (direct from PSUM) ---
            cand = p1c.tile([128, NCH * 8], F32, tag="cand")
            top32 = p1c.tile([128, 32], F32, tag="top32")
            for s in range(NQ):
                ps = psA.tile([128, 512], F32, tag="ps")
                nc.tensor.matmul(ps[:], xnROT[:, t * 128:(t + 1) * 128],
                                 xnROT[:, s * 512:(s + 1) * 512])
                for q in range(512 // CH):
                    cidx = (s * 512) // CH + q
                    nc.vector.max(cand[:, cidx * 8:(cidx + 1) * 8],
                                  ps[:, q * CH:(q + 1) * CH])
            # --- top-32 of candidates ---
            nc.vector.max(top32[:, 0:8], cand[:])
            for rnd in range(1, 4):
                nc.vector.match_replace(cand[:], top32[:, (rnd - 1) * 8:rnd * 8],
                                        cand[:], NEG_FILL)
                nc.vector.max(top32[:, rnd * 8:(rnd + 1) * 8], cand[:])
            # threshold (clamped at 0: raw-A masking == relu-A masking)
            nc.vector.tensor_scalar_max(thrM[:, t:t + 1], top32[:, 31:32], 0.0)
            # degree = sum(relu(top32)); max(max(v,0),v) == relu(v)
            dsc = p1c.tile([128, 32], F32, tag="dsc")
            nc.vector.scalar_tensor_tensor(dsc[:], top32[:], 0.0, top32[:],
                                           op0=Alu.max, op1=Alu.max,
                                           accum_out=degM[:, t:t + 1])
            # --- AT rebuild: 2 col-tiles per row-tile, halves for psum rot ---
            for c in (2 * t, 2 * t + 1):
                for h2 in range(2):
                    pat = psAT.tile([128, 1024], F32, tag="pat")
                    for s2 in range(2):
                        lo = h2 * 1024 + s2 * 512
                        nc.tensor.matmul(pat[:, s2 * 512:(s2 + 1) * 512],
                                         xnROT[:, c * 128:(c + 1) * 128],
                                         xnROT[:, lo:lo + 512])
                    nc.scalar.copy(MT[:, c, h2 * 1024:(h2 + 1) * 1024], pat[:])
            # threshold broadcast in two chunks on the idle gpsimd queue:
            # the 12-tile chunk's DMA roundtrip hides under pass-1, so the
            # mask drain only waits on the final 4-tile chunk's short chain.
            if t == RT - 5 or t == RT - 1:
                t0, t1 = (0, RT - 4) if t == RT - 5 else (RT - 4, RT)
                sl = bass.ds(t0 * 128, (t1 - t0) * 128)
                nc.gpsimd.dma_start(
                    thr_dram.ap()[sl].rearrange("(t p) -> p t", p=128),
                    thrM[:, t0:t1])
                nc.gpsimd.dma_start(THRb[:, t0 * 128:t1 * 128],
                                    thr_dram.ap()[sl].partition_broadcast(128))
            if t == RT - 1:
                emit_dm_chain()

    early_stack.close()  # xnT16/xnROT/h16g dead after pass 1

    # =====================================================================
    # Pass 2/3: Chebyshev products against MT
    # =====================================================================
    RG = max(r // 512, 1)
    RW = min(512, r)
    TPG = RW // 128

    late = stack.enter_context(tc.tile_pool(name="late", bufs=1))
    xres_all = late.tile([128, RT, DDIM], F32, tag="xres_all")
    P2a = late.tile([128, RT, HDIM], F16, tag="P2a")
    T1loc = late.tile([128, RT, HDIM], F16, tag="T1loc")
    # components for the y-stage matmuls, [h_p, local_row_f] layout (flat):
    hTa = late.tile([128, r], F16, tag="hTa")
    T1T = late.tile([128, r], F16, tag="T1T")
    QtaT = late.tile([128, r], F16, tag="QtaT")
    T2T = late.tile([128, r], F16, tag="T2T")
    # per-row coefficient vectors broadcast across partitions
    Q2b = late.tile([128, r], F16, tag="Q2b")
    CDM2b = late.tile([128, r], F16, tag="CDM2b")
    q2h = late.tile([128, RT], F16, tag="q2h")
    cdm2h = late.tile([128, RT], F16, tag="cdm2h")

    if cfg.b2:
        B2R = late.tile([128, ODIM], F32, tag="B2R")
        nc.sync.dma_start(B2R[:], b2_e.ap().partition_broadcast(128))

    def combine_pass(G, ps_pool, ps_tr_pool, sm_pool, which, strip_cb=None):
        """Chebyshev product with G: all RG strips accumulate over node tiles
        j as soon as MT[j] unmasks (overlaps the mask drain / T1 exchange)."""
        pstr = [ps_pool.tile([128, RW], F32, tag=f"pstr{rg}", name=f"pstr{rg}")
                for rg in range(RG)]
        for j in range(NT):
            for rg in range(RG):
                nc.tensor.matmul(pstr[rg][:], G[:, j, :],
                                 MT[:, j, rg * RW:(rg + 1) * RW],
                                 start=(j == 0), stop=(j == NT - 1))
        for rg in range(RG):
            raw16 = sm_pool.tile([128, RW], F16, tag="raw16")
            nc.scalar.copy(raw16[:], pstr[rg][:])
            if which == "T1":
                for ti in range(TPG):
                    t = rg * TPG + ti
                    ptr = ps_tr_pool.tile([128, HDIM], F16, tag="ptrC")
                    nc.tensor.transpose(ptr[:], raw16[:, ti * 128:(ti + 1) * 128],
                                        id16[:])
                    nc.vector.scalar_tensor_tensor(T1loc[:, t, :], ptr[:],
                                                   dmv[:, CDM1, t:t + 1],
                                                   P2a[:, t, :],
                                                   op0=Alu.mult, op1=Alu.add)
            else:
                # T2^T strip directly in [h, row] layout: no transposes.
                sl = slice(rg * RW, (rg + 1) * RW)
                nc.vector.tensor_tensor(T2T[:, sl], raw16[:], CDM2b[:, sl], Alu.mult)
                nc.vector.tensor_tensor(T2T[:, sl], T2T[:, sl], QtaT[:, sl], Alu.add)
            if strip_cb is not None:
                strip_cb(rg)

    # ---- pass 2: T1 ----
    t1iv = t1_in.ap().rearrange("(t p) d -> p t d", p=128)
    with tc.tile_pool(name="p2G", bufs=1) as p2G, \
         tc.tile_pool(name="p2s", bufs=4) as p2s, \
         tc.tile_pool(name="p2ps", bufs=1, space="PSUM") as p2ps, \
         tc.tile_pool(name="p2psT", bufs=3, space="PSUM") as p2psT:
        nc.scalar.copy(q2h[:], dmv[:, Q2, :])
        nc.scalar.copy(cdm2h[:], dmv[:, CDM2, :])
        nc.sync.dma_start(q2_dram.ap().rearrange("(t p) -> p t", p=128), q2h[:])
        nc.sync.dma_start(Q2b[:], q2_dram.ap().partition_broadcast(128))
        nc.sync.dma_start(cdm2_dram.ap().rearrange("(t p) -> p t", p=128), cdm2h[:])
        nc.sync.dma_start(CDM2b[:], cdm2_dram.ap().partition_broadcast(128))
        G = p2G.tile([128, NT, HDIM], F16, tag="G")
        for g in range(NT):
            nc.scalar.activation(G[:, g, :], hROT[:, g * 128:(g + 1) * 128],
                                 Act.Copy, scale=dm12rot[:, g:g + 1])
        for t in range(RT):
            nc.vector.tensor_scalar_mul(P2a[:, t, :], hROT[:, t * 128:(t + 1) * 128],
                                        dmv[:, Q1, t:t + 1])

        for t in range(RT):
            phx = p2psT.tile([128, HDIM], F16, tag="ptrC")
            nc.tensor.transpose(phx[:], hROT[:, t * 128:(t + 1) * 128], id16[:])
            nc.scalar.copy(hTa[:, t * 128:(t + 1) * 128], phx[:])

        # --- mask drain: MT[c] = (MT[c] >= THRb) * MT[c], fp16 2x TTs.
        # scalar_tensor_tensor has no DVE fast modes; tensor_tensor runs 2x
        # with all-fp16 operands. (Pool cannot run TensorTensor on real HW —
        # neuron_isa engine check rejects it — so the whole drain is DVE; the
        # T1 strip matmuls below consume MT[j] as each tile unmasks.)
        for c in range(NT):
            m01 = m01a if c % 2 == 0 else m01p
            nc.vector.tensor_tensor(m01[:], THRb[:], MT[:, c, :], Alu.is_le)
            nc.vector.tensor_tensor(MT[:, c, :], m01[:], MT[:, c, :], Alu.mult)

        combine_pass(G, p2ps, p2psT, p2s, "T1")
        nc.sync.dma_start(t1iv[:], T1loc[:])
        nc.sync.dma_start(xres_all[:], xm.ap().rearrange("(t p) d -> p t d", p=128))
        # precomputed while the T1 exchange is in flight: the y-stage's
        # h^T / T1^T component tiles and Qt^T = q2*T1^T - h^T (flat TTs).
        for t in range(RT):
            pT1 = p2psT.tile([128, HDIM], F16, tag="ptrC")
            nc.tensor.transpose(pT1[:], T1loc[:, t, :], id16[:])
            nc.scalar.copy(T1T[:, t * 128:(t + 1) * 128], pT1[:])
        nc.vector.tensor_tensor(QtaT[:], T1T[:], Q2b[:], Alu.mult)
        nc.vector.tensor_tensor(QtaT[:], QtaT[:], hTa[:], Alu.subtract)

    # T1 exchange
    if cfg.use_cc:
        nc.gpsimd.collective_compute("AllGather", Alu.bypass, replica_groups=groups,
                                     ins=[t1_in[:].opt()], outs=[t1_out[:].opt()])
    else:
        nc.sync.dma_start(t1_out[0:r, :], t1_in[:])
        if n > r:
            nc.sync.dma_start(t1_out[r:n, :], t1_in[:])

    # ---- pass 3: T2 combine ----
    with tc.tile_pool(name="p3G", bufs=1) as p3G, \
         tc.tile_pool(name="p3s", bufs=4) as p3s, \
         tc.tile_pool(name="p3ps", bufs=1, space="PSUM") as p3ps:
        G2 = p3G.tile([128, NT, HDIM], F16, tag="G2")
        # my half of G2 comes straight from local T1loc (no exchange dep):
        # the first RT node tiles of the T2 matmuls run during the exchange.
        for l in range(RT):
            nc.scalar.activation(G2[:, l, :], T1loc[:, l, :], Act.Copy,
                                 scale=dm12rot[:, l:l + 1])
        T1oth = p3G.tile([128, RT, HDIM], F16, tag="T1oth")
        t1ovr = t1_out.ap()[bass.ds(ooff, r), :].rearrange("(g p) d -> p g d", p=128)
        for g0 in range(0, RT, 4):
            nc.sync.dma_start(T1oth[:, g0:g0 + 4, :], t1ovr[:, g0:g0 + 4, :])
            for g in range(g0, g0 + 4):
                nc.scalar.activation(G2[:, RT + g, :], T1oth[:, g, :], Act.Copy,
                                     scale=dm12rot[:, RT + g:RT + g + 1])
        combine_pass(G2, p3ps, None, p3s, "T2")

    # ---- output stage: y = [h,T1,T2] @ w2 (+b2); out = x + tanh(gate)*y ----
    with tc.tile_pool(name="po", bufs=1) as po, \
         tc.tile_pool(name="popsY", bufs=3, space="PSUM") as popsY:
        outt_all = po.tile([128, RT, ODIM], F32, tag="outt_all")
        oev = out_e.ap().rearrange("(t p) d -> p t d", p=128)
        # quad-wide: 12 matmuls into one psum tile, ONE 4-wide combine TSP
        for t0 in range(0, RT, 4):
            py4 = popsY.tile([128, 4, ODIM], F32, tag="pyY4")
            for ti in range(4):
                t = t0 + ti
                sl = slice(t * 128, (t + 1) * 128)
                comps = [hTa[:, sl], T1T[:, sl], T2T[:, sl]]
                for k in range(KCHEB):
                    nc.tensor.matmul(py4[:, ti, :], comps[k][:], w2s[:, k, :],
                                     start=(k == 0), stop=(k == KCHEB - 1))
                if cfg.b2:
                    nc.vector.tensor_tensor(py4[:, ti, :], py4[:, ti, :],
                                            B2R[:], Alu.add)
            nc.vector.scalar_tensor_tensor(outt_all[:, t0:t0 + 4, :], py4[:], tg,
                                           xres_all[:, t0:t0 + 4, :],
                                           op0=Alu.mult, op1=Alu.add)
            nc.sync.dma_start(oev[:, t0:t0 + 4, :], outt_all[:, t0:t0 + 4, :])

    stack.close()


def build(cfg, num_devices):
    nc = bacc.Bacc("TRN2", target_bir_lowering=False, debug=False,
                   num_devices=num_devices)
    with tile.TileContext(nc) as tc:
        _emit(nc, tc, cfg)
    nc.compile()
    return nc


def _host_scalars(log_tau, gate):
    tau = max(float(np.exp(np.float32(log_tau))), 1e-3)
    c1 = (1.0 - TELEPORT) / tau
    c2 = (1.0 - TELEPORT) / tau + TELEPORT
    tg = float(np.tanh(np.float32(gate)))
    return c1, c2, tg


def _flags(ln_g, ln_b, b1, b2):
    return (not np.all(ln_g == 1.0), not np.all(ln_b == 0.0),
            not np.all(b1 == 0.0), not np.all(b2 == 0.0))


_CACHE = {}


def kernel(x, ln_g, ln_b, w1, b1, w2, b2, log_tau, gate):
    x = np.ascontiguousarray(x, dtype=np.float32)
    assert x.shape == (BSZ, NFULL, DDIM), x.shape
    scalars = _host_scalars(log_tau, gate)
    flags = _flags(np.asarray(ln_g), np.asarray(ln_b), np.asarray(b1), np.asarray(b2))
    key = (scalars, flags)
    if key not in _CACHE:
        cfg = Cfg(NFULL, NFULL // 2, True, scalars, flags)
        _CACHE[key] = (build(cfg, N_CORES), cfg)
    nc, cfg = _CACHE[key]

    r = cfg.r
    base = {
        "w1e": np.ascontiguousarray(w1, np.float32),
        "w2e": np.ascontiguousarray(w2, np.float32),
        "lng": np.ascontiguousarray(ln_g, np.float32),
        "lnb": np.ascontiguousarray(ln_b, np.float32),
        "b1e": np.ascontiguousarray(b1, np.float32),
        "b2e": np.ascontiguousarray(b2, np.float32),
    }
    in_maps = []
    for c in range(N_CORES):
        b, j = c // 2, c % 2
        m = dict(base)
        m["xf"] = x[b]
        m["xm"] = np.ascontiguousarray(x[b, j * r:(j + 1) * r, :])
        in_maps.append(m)

    res = run_bass_kernel_spmd(nc, in_maps, core_ids=list(range(N_CORES)))
    out = np.empty_like(x)
    for c in range(N_CORES):
        b, j = c // 2, c % 2
        out[b, j * r:(j + 1) * r, :] = res.results[c]["out"]
    return out



# revision 3
# speedup vs baseline: 1.1292x; 1.1292x over previous
"""Trainium2 Bass kernel for nn_AdaptiveChebBlock (8 NeuronCores).

Sharding: batch b = core//2 (4 batches), row-half j = core%2 (2048 rows each).
Each core computes its 2048 rows of the dynamic top-k adjacency + Chebyshev
propagation for its batch; pair collectives (AllGather over [2c,2c+1]) exchange
the degree vector and T1.

v3 design (vs. v2 fp16 mask-drain baseline):
- xn is fp8e4m3: all A-build matmuls (row-side candidate strips + AT side)
  run fp8. Candidates/threshold and the shifted adjacency derive from the
  SAME fp8 products, so the top-k mask stays self-consistent.
- shift decomposition instead of a DVE mask drain: the AT-side matmul
  accumulates a rank-1 fp16 term (ones^T x (-thr[row])) into the same PSUM,
  so PSUM holds A^T - thr[row]. The ACT PSUM->SBUF copy applies Relu and
  writes S = relu(A - thr) in fp8 (MT_S). Masked adjacency = S + thr*ind
  with ind = (S > 0), computed per element by one fp8 tensor_scalar (2x
  DVE mode, partially offloaded to the idle GPSIMD/Pool engine).
- Chebyshev combines are fp8 DoubleRow matmuls (j-tile pairs) against MT_S
  and IND with G = dm12*h in fp8: 4x fewer PE cycles than fp16. The per-row
  thr multiply happens post-contraction: T1T = CDM1b*psS + (CDM1*thr)b*psI
  + (c2*dm12^2*h)T, all flat [h,row] tensor_tensor ops.
- pass 1 interleaves per row-quarter: top-k (DVE Max) for rows of quarter q
  overlaps the AT build + relu copies + ind8 of quarter q-1, so ACT/Pool/PE
  work hides under the irreducible DVE Max scan.
"""
import os, sys
os.environ.setdefault("JAX_PLATFORMS", "")
for _p in ("/root/.axon_site/_ro/trn_rl_repo", "/opt/trn_rl_repo"):
    if os.path.isdir(_p):
        if _p not in sys.path:
            sys.path.insert(0, _p)
        break  # use exactly one copy — mixing versions breaks imports

import numpy as np

import concourse.bass as bass
import concourse.bacc as bacc
import concourse.tile as tile
import concourse.mybir as mybir
import concourse.masks as masks
from concourse.bass_utils import run_bass_kernel_spmd

F32 = mybir.dt.float32
F16 = mybir.dt.float16
F8 = mybir.dt.float8e4
Alu = mybir.AluOpType
Act = mybir.ActivationFunctionType
DRow = mybir.MatmulPerfMode.DoubleRow

KCHEB = 3
TOPK = 32
TELEPORT = 0.1
LN_EPS = 1e-5

# problem shape (hardcoded per spec)
BSZ, NFULL, DDIM = 4, 4096, 128
HDIM, ODIM = 128, 128
N_CORES = 8

NEG_FILL = -1.0e30


class Cfg:
    def __init__(self, n_nodes, n_rows, use_cc, scalars, flags, gelu=True,
                 pool_quarters=3):
        self.n = n_nodes            # nodes this core sees (columns of A)
        self.r = n_rows             # rows this core owns
        self.NT = n_nodes // 128    # node tiles
        self.RT = n_rows // 128     # row tiles
        self.use_cc = use_cc        # emit pair collectives (8-core mode)
        self.c1, self.c2, self.tg = scalars
        # flags: which optional affine params are non-trivial
        self.lng, self.lnb, self.b1, self.b2 = flags
        self.gelu = gelu            # False only for CoreSim (no Gelu in interp)
        self.pool_quarters = pool_quarters  # ind8 quarters on gpsimd


def _emit(nc, tc, cfg):
    """Emit the whole per-core graph inside TileContext tc."""
    n, r, NT, RT = cfg.n, cfg.r, cfg.NT, cfg.RT
    c1, c2, tg = cfg.c1, cfg.c2, cfg.tg
    gelu_f = Act.Gelu if cfg.gelu else Act.Identity

    # ---- DRAM I/O -------------------------------------------------------
    xf = nc.dram_tensor("xf", [n, DDIM], F32, kind="ExternalInput")       # full batch slice
    xm = nc.dram_tensor("xm", [r, DDIM], F32, kind="ExternalInput")       # my rows
    w1e = nc.dram_tensor("w1e", [DDIM, HDIM], F32, kind="ExternalInput")
    w2e = nc.dram_tensor("w2e", [KCHEB * HDIM, ODIM], F32, kind="ExternalInput")
    lng_e = nc.dram_tensor("lng", [DDIM], F32, kind="ExternalInput")
    lnb_e = nc.dram_tensor("lnb", [DDIM], F32, kind="ExternalInput")
    b1_e = nc.dram_tensor("b1e", [HDIM], F32, kind="ExternalInput")
    b2_e = nc.dram_tensor("b2e", [ODIM], F32, kind="ExternalInput")
    out_e = nc.dram_tensor("out", [r, ODIM], F32, kind="ExternalOutput")

    # DRAM scratch
    h16_dram = nc.dram_tensor("h16_scr", [128, n], F16)   # h fp16 staging for rotation
    xnt8_dram = nc.dram_tensor("xnt8_scr", [128, n], F8)  # xn^T fp8 staging for rotation
    thr_dram = nc.dram_tensor("thr_scr", [r], F16)
    nthr_dram = nc.dram_tensor("nthr_scr", [r], F16)
    cdm1_dram = nc.dram_tensor("cdm1_scr", [r], F16)
    q1_dram = nc.dram_tensor("q1_scr", [r], F16)
    q2_dram = nc.dram_tensor("q2_scr", [r], F16)
    cdm2_dram = nc.dram_tensor("cdm2_scr", [r], F16)
    dm_in = nc.dram_tensor("dm_in", [r], F32)
    t1_in = nc.dram_tensor("t1_in", [r, HDIM], F16)
    # NOTE: Shared addr_space is rejected for 2-rank groups; plain DRAM works.
    dm_out = nc.dram_tensor("dm_out", [n], F32)
    t1_out = nc.dram_tensor("t1_out", [n, HDIM], F16)
    groups = [[0, 1], [2, 3], [4, 5], [6, 7]]

    import contextlib
    stack = contextlib.ExitStack()
    const = stack.enter_context(tc.tile_pool(name="const", bufs=1))
    persist = stack.enter_context(tc.tile_pool(name="persist", bufs=1))

    id16 = const.tile([128, 128], F16, tag="id16")
    masks.make_identity(nc, id16[:])
    w1s16 = const.tile([DDIM, HDIM], F16, tag="w1s16")
    ones1 = const.tile([1, 128], F16, tag="ones1")
    nc.vector.memset(ones1[:], 1.0)
    if cfg.lng:
        LNG = const.tile([128, DDIM], F32, tag="LNG")
        nc.sync.dma_start(LNG[:], lng_e.ap().partition_broadcast(128))
    if cfg.lnb:
        LNB = const.tile([128, DDIM], F32, tag="LNB")
        nc.sync.dma_start(LNB[:], lnb_e.ap().partition_broadcast(128))
    if cfg.b1:
        B1R = const.tile([128, HDIM], F32, tag="B1R")
        nc.sync.dma_start(B1R[:], b1_e.ap().partition_broadcast(128))

    MT_S = persist.tile([128, NT, r], F8, tag="MT_S")     # relu(A - thr)^T, fp8
    IND = persist.tile([128, NT, r], F8, tag="IND")       # (S > 0), fp8
    w2s = persist.tile([128, KCHEB, ODIM], F16, tag="w2s")
    hROT = persist.tile([128, n], F16, tag="hROT")        # h fp16, rotated node order
    degM = persist.tile([128, RT], F32, tag="degM")
    dmv = persist.tile([128, 6, RT], F32, tag="dmv")      # [deg|dm12|cdm1|q1|q2|cdm2]
    dm12rot = persist.tile([128, NT], F32, tag="dm12rot")  # dm12, rotated node order
    thrM = persist.tile([128, RT], F16, tag="thrM")       # per-row-tile thresholds
    thrN = persist.tile([128, RT], F16, tag="thrN")       # negated thresholds

    # my row-window start (in nodes): core parity picks the half.
    # pid is a RUNTIME scalar: noff/ooff may only appear in DMA source offsets.
    pid = nc.partition_id()
    noff = (pid % 2) * (n - r)        # my half start
    ooff = ((pid + 1) % 2) * (n - r)  # other half start

    # =====================================================================
    # Feature pass: x tiles -> LN -> h=gelu(.@w1) -> h16, xn -> xn^T fp8
    # =====================================================================
    early_stack = contextlib.ExitStack()
    early = early_stack.enter_context(tc.tile_pool(name="early", bufs=1))
    xn8ROT = early.tile([128, n], F8, tag="xn8ROT")    # my half first
    negthr = early.tile([1, r], F16, tag="negthr")     # -thr by row, partition 0

    def _moments_arith(pool, nt, bnst, tagp, eps):
        """mean + 1/sqrt(var+eps) (+ sum-of-squares) from bn_stats output."""
        me, mo = bnst[:, :, 1], bnst[:, :, 4]
        m2e, m2o = bnst[:, :, 2], bnst[:, :, 5]
        mu = pool.tile([128, nt], F32, tag=tagp + "_mu", name="mu")
        rstd = pool.tile([128, nt], F32, tag=tagp + "_rstd", name="rstd")
        ssq = pool.tile([128, nt], F32, tag=tagp + "_ssq", name="ssq")
        dl = pool.tile([128, nt], F32, tag=tagp + "_dl", name="dl")
        nc.vector.tensor_tensor(dl[:], me, mo, Alu.subtract)
        nc.vector.tensor_tensor(dl[:], dl[:], dl[:], Alu.mult)       # delta^2
        nc.vector.tensor_tensor(mu[:], me, mo, Alu.add)
        nc.vector.tensor_scalar_mul(mu[:], mu[:], 0.5)               # mean
        nc.vector.tensor_tensor(rstd[:], m2e, m2o, Alu.add)
        nc.vector.scalar_tensor_tensor(rstd[:], dl[:], float(DDIM) / 4.0, rstd[:],
                                       op0=Alu.mult, op1=Alu.add)    # M2 total
        nc.vector.tensor_tensor(ssq[:], mu[:], mu[:], Alu.mult)
        nc.vector.scalar_tensor_tensor(ssq[:], ssq[:], float(DDIM), rstd[:],
                                       op0=Alu.mult, op1=Alu.add)    # sum sq
        nc.vector.tensor_scalar(rstd[:], rstd[:], 1.0 / DDIM, eps,
                                op0=Alu.mult, op1=Alu.add)           # var + eps
        nc.scalar.activation(rstd[:], rstd[:], Act.Sqrt)
        nc.vector.reciprocal(rstd[:], rstd[:])
        return mu, rstd, ssq

    def feature_pass(src, nt, pool, tpool, psum, psum2):
        xall = pool.tile([128, nt, DDIM], F32, tag="ff_xall")
        hall = pool.tile([128, nt, HDIM], F32, tag="ff_hall")
        bnx = pool.tile([128, nt, 6], F32, tag="ff_bnx")
        bnh = pool.tile([128, nt, 6], F32, tag="ff_bnh")
        for g0 in range(0, nt, 4):
            nc.sync.dma_start(
                xall[:, g0:g0 + 4, :],
                src.ap().rearrange("(g p) d -> p g d", p=128)[:, g0:g0 + 4, :])
        w1f = pool.tile([DDIM, HDIM], F32, tag="ff_w1f")
        nc.sync.dma_start(w1f[:], w1e[:])
        nc.scalar.copy(w1s16[:], w1f[:])
        # processed in quarter-blocks so each block's matmul/gelu/norm chain
        # overlaps the next block's stats instead of waiting on them
        nh = nt // 4
        for hf in range(4):
            lo = hf * nh
            for g in range(lo, lo + nh):
                nc.vector.bn_stats(bnx[:, g, :], xall[:, g, :])
            mu, rstd, _ = _moments_arith(pool, nh, bnx[:, lo:lo + nh, :],
                                         f"ffx{hf}", LN_EPS)
            for g0 in range(lo, lo + nh, 4):
                ph4 = psum2.tile([128, 4, 128], F32, tag="fp_ph4")
                ptx4 = psum.tile([128, 4, 128], F16, tag="fp_ptx4")
                xlnT4 = tpool.tile([128, 4, DDIM], F16, tag="fp_xlnT4")
                for gi in range(4):
                    g = g0 + gi
                    xln = tpool.tile([128, DDIM], F16, tag="fp_xln")
                    nc.vector.tensor_scalar(xln[:], xall[:, g, :],
                                            mu[:, g - lo:g - lo + 1],
                                            rstd[:, g - lo:g - lo + 1],
                                            op0=Alu.subtract, op1=Alu.mult)
                    if cfg.lng:
                        nc.vector.tensor_tensor(xln[:], xln[:], LNG[:], Alu.mult)
                    if cfg.lnb:
                        nc.vector.tensor_tensor(xln[:], xln[:], LNB[:], Alu.add)
                    nc.tensor.transpose(ptx4[:, gi, :], xln[:], id16[:])
                nc.scalar.copy(xlnT4[:], ptx4[:])
                for gi in range(4):
                    nc.tensor.matmul(ph4[:, gi, :], xlnT4[:, gi, :], w1s16[:])
                    if cfg.b1:
                        nc.vector.tensor_tensor(ph4[:, gi, :], ph4[:, gi, :],
                                                B1R[:], Alu.add)
                nc.scalar.activation(hall[:, g0:g0 + 4, :], ph4[:], gelu_f)
                h16s = tpool.tile([128, 4, HDIM], F16, tag="fp_h16s")
                nc.vector.tensor_copy(h16s[:], hall[:, g0:g0 + 4, :])
                nc.sync.dma_start(
                    h16_dram.ap().rearrange("p (g d) -> p g d", d=HDIM)[:, g0:g0 + 4, :],
                    h16s[:])
                for gi in range(4):
                    g = g0 + gi
                    nc.vector.bn_stats(bnh[:, g, :], hall[:, g, :])
            _, _, ssqh = _moments_arith(pool, nh, bnh[:, lo:lo + nh, :],
                                        f"ffh{hf}", 0.0)
            invh = pool.tile([128, nh], F32, tag=f"ff_invh{hf}", name="invh")
            nc.scalar.activation(invh[:], ssqh[:], Act.Sqrt)
            nc.vector.tensor_scalar_max(invh[:], invh[:], 1e-12)
            nc.vector.reciprocal(invh[:], invh[:])
            for g0 in range(lo, lo + nh, 4):
                xn4 = tpool.tile([128, 4, HDIM], F16, tag="fp_xn4")
                pt4 = psum.tile([128, 4, 128], F16, tag="fp_pt4")
                for gi in range(4):
                    g = g0 + gi
                    nc.vector.tensor_scalar_mul(xn4[:, gi, :], hall[:, g, :],
                                                invh[:, g - lo:g - lo + 1])
                    nc.tensor.transpose(pt4[:, gi, :], xn4[:, gi, :], id16[:])
                xns8 = tpool.tile([128, 4, 128], F8, tag="fp_xns8")
                nc.scalar.copy(xns8[:], pt4[:])
                nc.sync.dma_start(
                    xnt8_dram.ap().rearrange("p (g d) -> p g d", d=128)[:, g0:g0 + 4, :],
                    xns8[:])

    with tc.tile_pool(name="p0", bufs=1) as p0w, \
         tc.tile_pool(name="p0t", bufs=3) as p0t, \
         tc.tile_pool(name="p0ps", bufs=3, space="PSUM") as p0ps, \
         tc.tile_pool(name="p0ps2", bufs=2, space="PSUM") as p0ps2:
        feature_pass(xf, NT, p0w, p0t, p0ps, p0ps2)
        for k in range(KCHEB):
            w2f = p0w.tile([128, ODIM], F32, tag="w2f")
            nc.sync.dma_start(w2f[:], w2e[k * 128:(k + 1) * 128, :])
            nc.scalar.copy(w2s[:, k, :], w2f[:])

    # Rotate into "my half first" node order (runtime offsets, DMA only).
    nc.sync.dma_start(xn8ROT[:, 0:r], xnt8_dram.ap()[:, bass.ds(noff, r)])
    nc.scalar.dma_start(xn8ROT[:, r:n], xnt8_dram.ap()[:, bass.ds(ooff, r)])
    nc.gpsimd.dma_start(hROT[:, 0:r], h16_dram.ap()[:, bass.ds(noff, r)])
    nc.gpsimd.dma_start(hROT[:, r:n], h16_dram.ap()[:, bass.ds(ooff, r)])

    # =====================================================================
    # Pass 1: row-side candidates (DVE max8 from PSUM) + shifted AT build
    # =====================================================================
    NQ = n // 512                 # 512-wide strips per row tile
    DEG, DM, CDM1, Q1, Q2, CDM2 = range(6)
    QW = 512                      # AT build row-window (one quarter = 4 tiles)

    def emit_dm_chain():
        """deg -> dm12 vectors + fp16 stagings, pair exchange of dm12."""
        nc.vector.tensor_scalar(dmv[:, DEG, :], degM[:], c1, c2,
                                op0=Alu.mult, op1=Alu.add)
        nc.scalar.activation(dmv[:, DM, :], dmv[:, DEG, :], Act.Sqrt)
        nc.vector.reciprocal(dmv[:, DM, :], dmv[:, DM, :])
        nc.vector.tensor_scalar_mul(dmv[:, CDM1, :], dmv[:, DM, :], c1)
        nc.vector.tensor_tensor(dmv[:, Q1, :], dmv[:, DM, :], dmv[:, DM, :], Alu.mult)
        nc.vector.tensor_scalar_mul(dmv[:, Q2, :], dmv[:, Q1, :], 2.0 * c2)
        nc.vector.tensor_scalar_mul(dmv[:, Q1, :], dmv[:, Q1, :], c2)
        nc.vector.tensor_scalar_mul(dmv[:, CDM2, :], dmv[:, DM, :], 2.0 * c1)
        nc.sync.dma_start(dm_in.ap().rearrange("(t p) -> p t", p=128), dmv[:, DM, :])
        if cfg.use_cc:
            nc.gpsimd.collective_compute("AllGather", Alu.bypass,
                                         replica_groups=groups,
                                         ins=[dm_in[:].opt()], outs=[dm_out[:].opt()])
        else:
            nc.sync.dma_start(dm_out[0:r], dm_in[:])
            if n > r:
                nc.sync.dma_start(dm_out[r:n], dm_in[:])
        # dm12 in rotated node order: my half first
        nc.sync.dma_start(dm12rot[:, 0:RT],
                          dm_out.ap()[bass.ds(noff, r)].rearrange("(g p) -> p g", p=128))
        nc.sync.dma_start(dm12rot[:, RT:NT],
                          dm_out.ap()[bass.ds(ooff, r)].rearrange("(g p) -> p g", p=128))

    dve_ind = []   # deferred (col, lo) ind8 slices for the DVE tail

    with tc.tile_pool(name="p1c", bufs=2) as p1c, \
         tc.tile_pool(name="p1psA", bufs=3, space="PSUM") as psA, \
         tc.tile_pool(name="p1psT", bufs=3, space="PSUM") as psAT:
        for t in range(RT):
            # --- row-side A strips (fp8) + per-512-chunk max8 from PSUM ---
            cand = p1c.tile([128, NQ * 8], F32, tag="cand")
            top32 = p1c.tile([128, 32], F32, tag="top32")
            for s in range(NQ):
                ps = psA.tile([128, 512], F32, tag="ps")
                nc.tensor.matmul(ps[:], xn8ROT[:, t * 128:(t + 1) * 128],
                                 xn8ROT[:, s * 512:(s + 1) * 512])
                nc.vector.max(cand[:, s * 8:(s + 1) * 8], ps[:])
            # --- top-32 of candidates ---
            nc.vector.max(top32[:, 0:8], cand[:])
            for rnd in range(1, 4):
                nc.vector.match_replace(cand[:], top32[:, (rnd - 1) * 8:rnd * 8],
                                        cand[:], NEG_FILL)
                nc.vector.max(top32[:, rnd * 8:(rnd + 1) * 8], cand[:])
            # threshold (clamped at 0: raw-A masking == relu-A masking)
            nc.vector.tensor_scalar_max(thrM[:, t:t + 1], top32[:, 31:32], 0.0)
            nc.vector.tensor_scalar_mul(thrN[:, t:t + 1], thrM[:, t:t + 1], -1.0)
            # degree = sum(relu(top32)); max(max(v,0),v) == relu(v)
            dsc = p1c.tile([128, 32], F32, tag="dsc")
            nc.vector.scalar_tensor_tensor(dsc[:], top32[:], 0.0, top32[:],
                                           op0=Alu.max, op1=Alu.max,
                                           accum_out=degM[:, t:t + 1])
            if t % 4 != 3:
                continue
            # --- AT build for this row quarter: A^T - thr, relu'd to fp8 ---
            qi = t // 4
            lo = qi * QW
            sl = bass.ds(lo, QW)
            nc.sync.dma_start(
                nthr_dram.ap()[sl].rearrange("(t p) -> p t", p=128),
                thrN[:, qi * 4:(qi + 1) * 4])
            nc.sync.dma_start(negthr[0:1, lo:lo + QW], nthr_dram.ap()[sl])
            if t == RT - 1:
                # stage positive thresholds for the pass-2 broadcasts
                nc.sync.dma_start(
                    thr_dram.ap().rearrange("(t p) -> p t", p=128), thrM[:])
                emit_dm_chain()
            on_pool = qi < cfg.pool_quarters
            for c in range(NT):
                pat = psAT.tile([128, QW], F32, tag="pat")
                nc.tensor.matmul(pat[:], xn8ROT[:, c * 128:(c + 1) * 128],
                                 xn8ROT[:, lo:lo + QW], start=True, stop=False)
                nc.tensor.matmul(pat[:], ones1[:], negthr[0:1, lo:lo + QW],
                                 start=False, stop=True)
                nc.scalar.activation(MT_S[:, c, lo:lo + QW], pat[:], Act.Relu)
                if on_pool:
                    nc.gpsimd.tensor_scalar(IND[:, c, lo:lo + QW],
                                            MT_S[:, c, lo:lo + QW], 0.0, None,
                                            op0=Alu.is_gt)
                else:
                    dve_ind.append((c, lo))

    early_stack.close()  # xn8ROT/negthr dead after pass 1

    # =====================================================================
    # Pass 2/3: Chebyshev combines against MT_S/IND (fp8 DoubleRow)
    # =====================================================================
    late = stack.enter_context(tc.tile_pool(name="late", bufs=1))
    THRb = late.tile([128, r], F16, tag="THRb")     # becomes CT1b in place
    CDM1b = late.tile([128, r], F16, tag="CDM1b")
    Q1b = late.tile([128, r], F16, tag="Q1b")       # becomes Qh in place
    Q2b = late.tile([128, r], F16, tag="Q2b")       # becomes QtaT in place
    CDM2b = late.tile([128, r], F16, tag="CDM2b")
    CT2b = late.tile([128, r], F16, tag="CT2b")
    hTa = late.tile([128, r], F16, tag="hTa")
    T1T = late.tile([128, r], F16, tag="T1T")
    T2T = late.tile([128, r], F16, tag="T2T")
    T1loc = late.tile([128, RT, HDIM], F16, tag="T1loc")
    G8 = late.tile([128, NT, HDIM], F8, tag="G8")
    G28 = late.tile([128, NT, HDIM], F8, tag="G28")
    xres16 = late.tile([128, RT, DDIM], F16, tag="xres16")

    if cfg.b2:
        B2R = late.tile([128, ODIM], F32, tag="B2R")
        nc.sync.dma_start(B2R[:], b2_e.ap().partition_broadcast(128))

    def combine_pass(G, MT, which, psC, p2s):
        """Shifted Chebyshev product: psS = S^T x G, psI = ind^T x G per
        512-row strip (fp8 DoubleRow over j-tile pairs), then flat [h,row]
        assembly with the per-row coefficient broadcasts."""
        if which == "T1":
            CAb, CIb, ADDb = CDM1b, THRb, Q1b       # THRb==CT1b, Q1b==Qh here
        else:
            CAb, CIb, ADDb = CDM2b, CT2b, Q2b       # Q2b==QtaT here
        TT = T1T if which == "T1" else T2T
        for w in range(2):
            pss = []
            for rg in (2 * w, 2 * w + 1):
                psS = psC.tile([128, 512], F32, tag=f"psS{rg % 2}", name=f"psS{rg}")
                psI = psC.tile([128, 512], F32, tag=f"psI{rg % 2}", name=f"psI{rg}")
                pss.append((rg, psS, psI))
            for jp in range(NT // 2):
                st, sp = jp == 0, jp == NT // 2 - 1
                for rg, psS, psI in pss:
                    rsl = slice(rg * 512, (rg + 1) * 512)
                    nc.tensor.matmul(psS[:], G[:, 2 * jp:2 * jp + 2, :],
                                     MT[0][:, 2 * jp:2 * jp + 2, rsl],
                                     start=st, stop=sp, perf_mode=DRow)
                    nc.tensor.matmul(psI[:], G[:, 2 * jp:2 * jp + 2, :],
                                     MT[1][:, 2 * jp:2 * jp + 2, rsl],
                                     start=st, stop=sp, perf_mode=DRow)
            for rg, psS, psI in pss:
                rsl = slice(rg * 512, (rg + 1) * 512)
                u = p2s.tile([128, 512], F16, tag="u")
                v = p2s.tile([128, 512], F16, tag="v")
                nc.vector.tensor_tensor(u[:], psS[:], CAb[:, rsl], Alu.mult)
                nc.vector.tensor_tensor(v[:], psI[:], CIb[:, rsl], Alu.mult)
                nc.vector.tensor_tensor(u[:], u[:], v[:], Alu.add)
                nc.vector.tensor_tensor(TT[:, rsl], u[:], ADDb[:, rsl], Alu.add)

    # ---- pass 2: T1 ----
    t1iv = t1_in.ap().rearrange("(t p) d -> p t d", p=128)
    with tc.tile_pool(name="p2s", bufs=4) as p2s, \
         tc.tile_pool(name="p2ps", bufs=1, space="PSUM") as p2ps, \
         tc.tile_pool(name="p2psT", bufs=3, space="PSUM") as p2psT:
        # fp16 stagings of the per-row coefficient vectors -> broadcasts
        qstage = p2s.tile([128, 4, RT], F16, tag="qstage")
        for i, (row, dram) in enumerate([(CDM1, cdm1_dram), (Q1, q1_dram),
                                         (Q2, q2_dram), (CDM2, cdm2_dram)]):
            nc.scalar.copy(qstage[:, i, :], dmv[:, row, :])
            nc.sync.dma_start(dram.ap().rearrange("(t p) -> p t", p=128),
                              qstage[:, i, :])
        nc.sync.dma_start(CDM1b[:], cdm1_dram.ap().partition_broadcast(128))
        nc.sync.dma_start(Q1b[:], q1_dram.ap().partition_broadcast(128))
        nc.sync.dma_start(Q2b[:], q2_dram.ap().partition_broadcast(128))
        nc.sync.dma_start(CDM2b[:], cdm2_dram.ap().partition_broadcast(128))
        nc.sync.dma_start(THRb[:], thr_dram.ap().partition_broadcast(128))

        # deferred ind8 slices on DVE (fills the deg-exchange gap)
        for c, lo in dve_ind:
            nc.vector.tensor_scalar(IND[:, c, lo:lo + QW],
                                    MT_S[:, c, lo:lo + QW], 0.0, None,
                                    op0=Alu.is_gt)

        # G = dm12 * h in fp8 (2x_2p tensor_scalar)
        for g in range(NT):
            nc.vector.tensor_scalar_mul(G8[:, g, :], hROT[:, g * 128:(g + 1) * 128],
                                        dm12rot[:, g:g + 1])
        # h^T strips for the y-stage and the Qh/QtaT combines
        for t in range(RT):
            phx = p2psT.tile([128, HDIM], F16, tag="ptrC")
            nc.tensor.transpose(phx[:], hROT[:, t * 128:(t + 1) * 128], id16[:])
            nc.scalar.copy(hTa[:, t * 128:(t + 1) * 128], phx[:])
        # coefficient fixups (order matters: CT2b before THRb is overwritten)
        nc.vector.tensor_tensor(CT2b[:], CDM2b[:], THRb[:], Alu.mult)
        nc.vector.tensor_tensor(THRb[:], THRb[:], CDM1b[:], Alu.mult)  # CT1b
        nc.vector.tensor_tensor(Q1b[:], Q1b[:], hTa[:], Alu.mult)      # Qh

        combine_pass(G8, (MT_S, IND), "T1", p2ps, p2s)

        # T1 row-major (for G2 + exchange) via transposes
        for t in range(RT):
            pT1 = p2psT.tile([128, HDIM], F16, tag="ptrC")
            nc.tensor.transpose(pT1[:], T1T[:, t * 128:(t + 1) * 128], id16[:])
            nc.scalar.copy(T1loc[:, t, :], pT1[:])
        nc.sync.dma_start(t1iv[:], T1loc[:])
        nc.gpsimd.dma_start(xres16[:], xm.ap().rearrange("(t p) d -> p t d", p=128))
        # QtaT = q2*T1T - hTa, prebuilt while the T1 exchange is in flight
        nc.vector.tensor_tensor(Q2b[:], Q2b[:], T1T[:], Alu.mult)
        nc.vector.tensor_tensor(Q2b[:], Q2b[:], hTa[:], Alu.subtract)

    # T1 exchange
    if cfg.use_cc:
        nc.gpsimd.collective_compute("AllGather", Alu.bypass, replica_groups=groups,
                                     ins=[t1_in[:].opt()], outs=[t1_out[:].opt()])
    else:
        nc.sync.dma_start(t1_out[0:r, :], t1_in[:])
        if n > r:
            nc.sync.dma_start(t1_out[r:n, :], t1_in[:])

    # ---- pass 3: T2 combine ----
    with tc.tile_pool(name="p3G", bufs=1) as p3G, \
         tc.tile_pool(name="p3s", bufs=4) as p3s, \
         tc.tile_pool(name="p3ps", bufs=1, space="PSUM") as p3ps:
        # my half of G2 comes straight from local T1loc (no exchange dep):
        # the first strips of the T2 matmuls run during the exchange.
        for l in range(RT):
            nc.vector.tensor_scalar_mul(G28[:, l, :], T1loc[:, l, :],
                                        dm12rot[:, l:l + 1])
        T1oth = p3G.tile([128, RT, HDIM], F16, tag="T1oth")
        t1ovr = t1_out.ap()[bass.ds(ooff, r), :].rearrange("(g p) d -> p g d", p=128)
        for g0 in range(0, RT, 4):
            nc.sync.dma_start(T1oth[:, g0:g0 + 4, :], t1ovr[:, g0:g0 + 4, :])
            for g in range(g0, g0 + 4):
                nc.vector.tensor_scalar_mul(G28[:, RT + g, :], T1oth[:, g, :],
                                            dm12rot[:, RT + g:RT + g + 1])
        combine_pass(G28, (MT_S, IND), "T2", p3ps, p3s)

    # ---- output stage: y = [h,T1,T2] @ w2 (+b2); out = x + tanh(gate)*y ----
    with tc.tile_pool(name="po", bufs=1) as po, \
         tc.tile_pool(name="popsY", bufs=3, space="PSUM") as popsY:
        outt_all = po.tile([128, RT, ODIM], F32, tag="outt_all")
        oev = out_e.ap().rearrange("(t p) d -> p t d", p=128)
        # quad-wide: 12 matmuls into one psum tile, ONE 4-wide combine TSP
        for t0 in range(0, RT, 4):
            py4 = popsY.tile([128, 4, ODIM], F32, tag="pyY4")
            for ti in range(4):
                t = t0 + ti
                sl = slice(t * 128, (t + 1) * 128)
                comps = [hTa[:, sl], T1T[:, sl], T2T[:, sl]]
                for k in range(KCHEB):
                    nc.tensor.matmul(py4[:, ti, :], comps[k][:], w2s[:, k, :],
                                     start=(k == 0), stop=(k == KCHEB - 1))
                if cfg.b2:
                    nc.vector.tensor_tensor(py4[:, ti, :], py4[:, ti, :],
                                            B2R[:], Alu.add)
            nc.vector.scalar_tensor_tensor(outt_all[:, t0:t0 + 4, :], py4[:], tg,
                                           xres16[:, t0:t0 + 4, :],
                                           op0=Alu.mult, op1=Alu.add)
            nc.sync.dma_start(oev[:, t0:t0 + 4, :], outt_all[:, t0:t0 + 4, :])

    stack.close()


def build(cfg, num_devices):
    nc = bacc.Bacc("TRN2", target_bir_lowering=False, debug=False,
                   num_devices=num_devices)
    with tile.TileContext(nc) as tc:
        _emit(nc, tc, cfg)
    nc.compile()
    return nc


def _host_scalars(log_tau, gate):
    tau = max(float(np.exp(np.float32(log_tau))), 1e-3)
    c1 = (1.0 - TELEPORT) / tau
    c2 = (1.0 - TELEPORT) / tau + TELEPORT
    tg = float(np.tanh(np.float32(gate)))
    return c1, c2, tg


def _flags(ln_g, ln_b, b1, b2):
    return (not np.all(ln_g == 1.0), not np.all(ln_b == 0.0),
            not np.all(b1 == 0.0), not np.all(b2 == 0.0))


_CACHE = {}


def kernel(x, ln_g, ln_b, w1, b1, w2, b2, log_tau, gate):
    x = np.ascontiguousarray(x, dtype=np.float32)
    assert x.shape == (BSZ, NFULL, DDIM), x.shape
    scalars = _host_scalars(log_tau, gate)
    flags = _flags(np.asarray(ln_g), np.asarray(ln_b), np.asarray(b1), np.asarray(b2))
    key = (scalars, flags)
    if key not in _CACHE:
        cfg = Cfg(NFULL, NFULL // 2, True, scalars, flags)
        _CACHE[key] = (build(cfg, N_CORES), cfg)
    nc, cfg = _CACHE[key]

    r = cfg.r
    base = {
        "w1e": np.ascontiguousarray(w1, np.float32),
        "w2e": np.ascontiguousarray(w2, np.float32),
        "lng": np.ascontiguousarray(ln_g, np.float32),
        "lnb": np.ascontiguousarray(ln_b, np.float32),
        "b1e": np.ascontiguousarray(b1, np.float32),
        "b2e": np.ascontiguousarray(b2, np.float32),
    }
    in_maps = []
    for c in range(N_CORES):
        b, j = c // 2, c % 2
        m = dict(base)
        m["xf"] = x[b]
        m["xm"] = np.ascontiguousarray(x[b, j * r:(j + 1) * r, :])
        in_maps.append(m)

    res = run_bass_kernel_spmd(nc, in_maps, core_ids=list(range(N_CORES)))
    out = np.empty_like(x)
    for c in range(N_CORES):
        b, j = c // 2, c % 2
        out[b, j * r:(j + 1) * r, :] = res.results[c]["out"]
    return out


# revision 14
# speedup vs baseline: 1.1560x; 1.0237x over previous
"""Trainium2 Bass kernel for nn_AdaptiveChebBlock (8 NeuronCores).

Sharding: batch b = core//2 (4 batches), row-half j = core%2 (2048 rows each).
Each core computes its 2048 rows of the dynamic top-k adjacency + Chebyshev
propagation for its batch; pair collectives (AllGather over [2c,2c+1]) exchange
the degree vector and T1.

v3 design (vs. v2 fp16 mask-drain baseline):
- xn is fp8e4m3: all A-build matmuls (row-side candidate strips + AT side)
  run fp8. Candidates/threshold and the shifted adjacency derive from the
  SAME fp8 products, so the top-k mask stays self-consistent.
- shift decomposition instead of a DVE mask drain: the AT-side matmul
  accumulates a rank-1 fp16 term (ones^T x (-thr[row])) into the same PSUM,
  so PSUM holds A^T - thr[row]. The ACT PSUM->SBUF copy applies Relu and
  writes S = relu(A - thr) in fp8 (MT_S). Masked adjacency = S + thr*ind
  with ind = (S > 0), computed per element by one fp8 tensor_scalar (2x
  DVE mode, partially offloaded to the idle GPSIMD/Pool engine).
- Chebyshev combines are fp8 DoubleRow matmuls (j-tile pairs) against MT_S
  and IND with G = dm12*h in fp8: 4x fewer PE cycles than fp16. The per-row
  thr multiply happens post-contraction: T1T = CDM1b*psS + (CDM1*thr)b*psI
  + (c2*dm12^2*h)T, all flat [h,row] tensor_tensor ops.
- pass 1 interleaves per row-quarter: top-k (DVE Max) for rows of quarter q
  overlaps the AT build + relu copies + ind8 of quarter q-1, so ACT/Pool/PE
  work hides under the irreducible DVE Max scan.
"""
import os, sys
os.environ.setdefault("JAX_PLATFORMS", "")
for _p in ("/root/.axon_site/_ro/trn_rl_repo", "/opt/trn_rl_repo"):
    if os.path.isdir(_p):
        if _p not in sys.path:
            sys.path.insert(0, _p)
        break  # use exactly one copy — mixing versions breaks imports

import numpy as np

import concourse.bass as bass
import concourse.bacc as bacc
import concourse.tile as tile
import concourse.mybir as mybir
import concourse.masks as masks
from concourse.bass_utils import run_bass_kernel_spmd

F32 = mybir.dt.float32
F16 = mybir.dt.float16
F8 = mybir.dt.float8e4
Alu = mybir.AluOpType
Act = mybir.ActivationFunctionType
DRow = mybir.MatmulPerfMode.DoubleRow

KCHEB = 3
TOPK = 32
TELEPORT = 0.1
LN_EPS = 1e-5

# problem shape (hardcoded per spec)
BSZ, NFULL, DDIM = 4, 4096, 128
HDIM, ODIM = 128, 128
N_CORES = 8

NEG_FILL = -1.0e30


class Cfg:
    def __init__(self, n_nodes, n_rows, use_cc, scalars, flags, gelu=True,
                 pool_quarters=3):
        self.n = n_nodes            # nodes this core sees (columns of A)
        self.r = n_rows             # rows this core owns
        self.NT = n_nodes // 128    # node tiles
        self.RT = n_rows // 128     # row tiles
        self.use_cc = use_cc        # emit pair collectives (8-core mode)
        self.c1, self.c2, self.tg = scalars
        # flags: which optional affine params are non-trivial
        self.lng, self.lnb, self.b1, self.b2 = flags
        self.gelu = gelu            # False only for CoreSim (no Gelu in interp)
        self.pool_quarters = pool_quarters  # ind8 quarters on gpsimd


def _emit(nc, tc, cfg):
    """Emit the whole per-core graph inside TileContext tc."""
    n, r, NT, RT = cfg.n, cfg.r, cfg.NT, cfg.RT
    c1, c2, tg = cfg.c1, cfg.c2, cfg.tg
    gelu_f = Act.Gelu if cfg.gelu else Act.Identity

    # ---- DRAM I/O -------------------------------------------------------
    xf = nc.dram_tensor("xf", [n, DDIM], F32, kind="ExternalInput")       # full batch slice
    xm = nc.dram_tensor("xm", [r, DDIM], F32, kind="ExternalInput")       # my rows
    w1e = nc.dram_tensor("w1e", [DDIM, HDIM], F32, kind="ExternalInput")
    w2e = nc.dram_tensor("w2e", [KCHEB * HDIM, ODIM], F32, kind="ExternalInput")
    lng_e = nc.dram_tensor("lng", [DDIM], F32, kind="ExternalInput")
    lnb_e = nc.dram_tensor("lnb", [DDIM], F32, kind="ExternalInput")
    b1_e = nc.dram_tensor("b1e", [HDIM], F32, kind="ExternalInput")
    b2_e = nc.dram_tensor("b2e", [ODIM], F32, kind="ExternalInput")
    out_e = nc.dram_tensor("out", [r, ODIM], F32, kind="ExternalOutput")

    # DRAM scratch
    thr_dram = nc.dram_tensor("thr_scr", [r], F16)
    cdm1_dram = nc.dram_tensor("cdm1_scr", [r], F16)
    q1_dram = nc.dram_tensor("q1_scr", [r], F16)
    q2_dram = nc.dram_tensor("q2_scr", [r], F16)
    cdm2_dram = nc.dram_tensor("cdm2_scr", [r], F16)
    dm_in = nc.dram_tensor("dm_in", [r], F32)
    t1_in = nc.dram_tensor("t1_in", [r, HDIM], F16)
    # NOTE: Shared addr_space is rejected for 2-rank groups; plain DRAM works.
    dm_out = nc.dram_tensor("dm_out", [n], F32)
    t1_out = nc.dram_tensor("t1_out", [n, HDIM], F16)
    groups = [[0, 1], [2, 3], [4, 5], [6, 7]]

    import contextlib
    stack = contextlib.ExitStack()
    const = stack.enter_context(tc.tile_pool(name="const", bufs=1))
    persist = stack.enter_context(tc.tile_pool(name="persist", bufs=1))

    id16 = const.tile([128, 128], F16, tag="id16")
    masks.make_identity(nc, id16[:])
    w1s16 = const.tile([DDIM, HDIM], F16, tag="w1s16")
    ones1 = const.tile([1, 128], F16, tag="ones1")
    nc.vector.memset(ones1[:], 1.0)
    if cfg.lng:
        LNG = const.tile([128, DDIM], F32, tag="LNG")
        nc.sync.dma_start(LNG[:], lng_e.ap().partition_broadcast(128))
    if cfg.lnb:
        LNB = const.tile([128, DDIM], F32, tag="LNB")
        nc.sync.dma_start(LNB[:], lnb_e.ap().partition_broadcast(128))
    if cfg.b1:
        B1R = const.tile([128, HDIM], F32, tag="B1R")
        nc.sync.dma_start(B1R[:], b1_e.ap().partition_broadcast(128))

    MT_S = persist.tile([128, NT, r], F8, tag="MT_S")     # relu(A - thr)^T, fp8
    IND = persist.tile([128, NT, r], F8, tag="IND")       # (S > 0), fp8
    w2s = persist.tile([128, KCHEB, ODIM], F16, tag="w2s")
    hROT = persist.tile([128, NT, 128], F16, tag="hROT")  # h fp16, rotated node order
    degM = persist.tile([128, RT], F32, tag="degM")
    dmv = persist.tile([128, 6, RT], F32, tag="dmv")      # [deg|dm12|cdm1|q1|q2|cdm2]
    dm12rot = persist.tile([128, NT], F32, tag="dm12rot")  # dm12, rotated node order
    thrM = persist.tile([128, RT], F16, tag="thrM")       # per-row-tile thresholds

    # my row-window start (in nodes): core parity picks the half.
    # pid is a RUNTIME scalar: noff/ooff may only appear in DMA source offsets.
    pid = nc.partition_id()
    noff = (pid % 2) * (n - r)        # my half start
    ooff = ((pid + 1) % 2) * (n - r)  # other half start

    # =====================================================================
    # Feature pass: x tiles -> LN -> h=gelu(.@w1) -> h16, xn -> xn^T fp8
    # =====================================================================
    early_stack = contextlib.ExitStack()
    early = early_stack.enter_context(tc.tile_pool(name="early", bufs=1))
    xn8ROT = early.tile([128, NT, 128], F8, tag="xn8ROT")  # my half first
    negthr = early.tile([1, r], F16, tag="negthr")     # -thr by row, partition 0

    def _moments_arith(pool, nt, bnst, tagp, eps):
        """mean + 1/sqrt(var+eps) (+ sum-of-squares) from bn_stats output."""
        me, mo = bnst[:, :, 1], bnst[:, :, 4]
        m2e, m2o = bnst[:, :, 2], bnst[:, :, 5]
        mu = pool.tile([128, nt], F32, tag=tagp + "_mu", name="mu")
        rstd = pool.tile([128, nt], F32, tag=tagp + "_rstd", name="rstd")
        ssq = pool.tile([128, nt], F32, tag=tagp + "_ssq", name="ssq")
        dl = pool.tile([128, nt], F32, tag=tagp + "_dl", name="dl")
        nc.vector.tensor_tensor(dl[:], me, mo, Alu.subtract)
        nc.vector.tensor_tensor(dl[:], dl[:], dl[:], Alu.mult)       # delta^2
        nc.vector.tensor_tensor(mu[:], me, mo, Alu.add)
        nc.vector.tensor_scalar_mul(mu[:], mu[:], 0.5)               # mean
        nc.vector.tensor_tensor(rstd[:], m2e, m2o, Alu.add)
        nc.vector.scalar_tensor_tensor(rstd[:], dl[:], float(DDIM) / 4.0, rstd[:],
                                       op0=Alu.mult, op1=Alu.add)    # M2 total
        nc.vector.tensor_tensor(ssq[:], mu[:], mu[:], Alu.mult)
        nc.vector.scalar_tensor_tensor(ssq[:], ssq[:], float(DDIM), rstd[:],
                                       op0=Alu.mult, op1=Alu.add)    # sum sq
        nc.vector.tensor_scalar(rstd[:], rstd[:], 1.0 / DDIM, eps,
                                op0=Alu.mult, op1=Alu.add)           # var + eps
        nc.scalar.activation(rstd[:], rstd[:], Act.Sqrt)
        nc.vector.reciprocal(rstd[:], rstd[:])
        return mu, rstd, ssq

    def feature_pass(src, nt, pool, tpool, psum, psum2):
        """Node features in ROTATED group order (my half first): the x loads
        use runtime ds() offsets so hROT/xn8ROT are written directly."""
        xall = pool.tile([128, nt, DDIM], F32, tag="ff_xall")
        hall = pool.tile([128, nt, HDIM], F32, tag="ff_hall")
        bnx = pool.tile([128, nt, 6], F32, tag="ff_bnx")
        bnh = pool.tile([128, nt, 6], F32, tag="ff_bnh")
        hg = nt // 2
        for g0 in range(0, nt, 4):
            off, gl = (noff, g0) if g0 < hg else (ooff, g0 - hg)
            nc.sync.dma_start(
                xall[:, g0:g0 + 4, :],
                src.ap()[bass.ds(off, r), :]
                .rearrange("(g p) d -> p g d", p=128)[:, gl:gl + 4, :])
        w1f = pool.tile([DDIM, HDIM], F32, tag="ff_w1f")
        nc.sync.dma_start(w1f[:], w1e[:])
        nc.scalar.copy(w1s16[:], w1f[:])
        # processed in quarter-blocks so each block's matmul/gelu/norm chain
        # overlaps the next block's stats instead of waiting on them
        nh = nt // 4
        for hf in range(4):
            lo = hf * nh
            for g in range(lo, lo + nh):
                nc.vector.bn_stats(bnx[:, g, :], xall[:, g, :])
            mu, rstd, _ = _moments_arith(pool, nh, bnx[:, lo:lo + nh, :],
                                         f"ffx{hf}", LN_EPS)
            for g0 in range(lo, lo + nh, 4):
                ph4 = psum2.tile([128, 4, 128], F32, tag="fp_ph4")
                ptx4 = psum.tile([128, 4, 128], F16, tag="fp_ptx4")
                xlnT4 = tpool.tile([128, 4, DDIM], F16, tag="fp_xlnT4")
                for gi in range(4):
                    g = g0 + gi
                    xln = tpool.tile([128, DDIM], F16, tag="fp_xln")
                    nc.vector.tensor_scalar(xln[:], xall[:, g, :],
                                            mu[:, g - lo:g - lo + 1],
                                            rstd[:, g - lo:g - lo + 1],
                                            op0=Alu.subtract, op1=Alu.mult)
                    if cfg.lng:
                        nc.vector.tensor_tensor(xln[:], xln[:], LNG[:], Alu.mult)
                    if cfg.lnb:
                        nc.vector.tensor_tensor(xln[:], xln[:], LNB[:], Alu.add)
                    nc.tensor.transpose(ptx4[:, gi, :], xln[:], id16[:])
                nc.scalar.copy(xlnT4[:], ptx4[:])
                for gi in range(4):
                    nc.tensor.matmul(ph4[:, gi, :], xlnT4[:, gi, :], w1s16[:])
                    if cfg.b1:
                        nc.vector.tensor_tensor(ph4[:, gi, :], ph4[:, gi, :],
                                                B1R[:], Alu.add)
                nc.scalar.activation(hall[:, g0:g0 + 4, :], ph4[:], gelu_f)
                nc.vector.tensor_copy(hROT[:, g0:g0 + 4, :], hall[:, g0:g0 + 4, :])
                for gi in range(4):
                    g = g0 + gi
                    nc.vector.bn_stats(bnh[:, g, :], hall[:, g, :])
            _, _, ssqh = _moments_arith(pool, nh, bnh[:, lo:lo + nh, :],
                                        f"ffh{hf}", 0.0)
            invh = pool.tile([128, nh], F32, tag=f"ff_invh{hf}", name="invh")
            nc.scalar.activation(invh[:], ssqh[:], Act.Sqrt)
            nc.vector.tensor_scalar_max(invh[:], invh[:], 1e-12)
            nc.vector.reciprocal(invh[:], invh[:])
            for g0 in range(lo, lo + nh, 4):
                xn4 = tpool.tile([128, 4, HDIM], F16, tag="fp_xn4")
                pt4 = psum.tile([128, 4, 128], F16, tag="fp_pt4")
                for gi in range(4):
                    g = g0 + gi
                    nc.vector.tensor_scalar_mul(xn4[:, gi, :], hall[:, g, :],
                                                invh[:, g - lo:g - lo + 1])
                    nc.tensor.transpose(pt4[:, gi, :], xn4[:, gi, :], id16[:])
                nc.scalar.copy(xn8ROT[:, g0:g0 + 4, :], pt4[:])

    with tc.tile_pool(name="p0", bufs=1) as p0w, \
         tc.tile_pool(name="p0t", bufs=3) as p0t, \
         tc.tile_pool(name="p0ps", bufs=3, space="PSUM") as p0ps, \
         tc.tile_pool(name="p0ps2", bufs=2, space="PSUM") as p0ps2:
        feature_pass(xf, NT, p0w, p0t, p0ps, p0ps2)
        for k in range(KCHEB):
            w2f = p0w.tile([128, ODIM], F32, tag="w2f")
            nc.sync.dma_start(w2f[:], w2e[k * 128:(k + 1) * 128, :])
            nc.scalar.copy(w2s[:, k, :], w2f[:])

    # =====================================================================
    # Pass 1: row-side candidates (DVE max8 from PSUM) + shifted AT build
    # =====================================================================
    NQ = n // 512                 # 512-wide strips per row tile
    DEG, DM, CDM1, Q1, Q2, CDM2 = range(6)
    QW = 512                      # AT build row-window (one quarter = 4 tiles)

    def emit_dm_chain():
        """deg -> dm12 vectors + fp16 stagings, pair exchange of dm12."""
        nc.vector.tensor_scalar(dmv[:, DEG, :], degM[:], c1, c2,
                                op0=Alu.mult, op1=Alu.add)
        nc.scalar.activation(dmv[:, DM, :], dmv[:, DEG, :], Act.Sqrt)
        nc.vector.reciprocal(dmv[:, DM, :], dmv[:, DM, :])
        nc.vector.tensor_scalar_mul(dmv[:, CDM1, :], dmv[:, DM, :], c1)
        nc.vector.tensor_tensor(dmv[:, Q1, :], dmv[:, DM, :], dmv[:, DM, :], Alu.mult)
        nc.vector.tensor_scalar_mul(dmv[:, Q2, :], dmv[:, Q1, :], 2.0 * c2)
        nc.vector.tensor_scalar_mul(dmv[:, Q1, :], dmv[:, Q1, :], c2)
        nc.vector.tensor_scalar_mul(dmv[:, CDM2, :], dmv[:, DM, :], 2.0 * c1)
        nc.sync.dma_start(dm_in.ap().rearrange("(t p) -> p t", p=128), dmv[:, DM, :])
        if cfg.use_cc:
            nc.gpsimd.collective_compute("AllGather", Alu.bypass,
                                         replica_groups=groups,
                                         ins=[dm_in[:].opt()], outs=[dm_out[:].opt()])
        else:
            nc.sync.dma_start(dm_out[0:r], dm_in[:])
            if n > r:
                nc.sync.dma_start(dm_out[r:n], dm_in[:])
        # dm12 in rotated node order: my half first
        nc.sync.dma_start(dm12rot[:, 0:RT],
                          dm_out.ap()[bass.ds(noff, r)].rearrange("(g p) -> p g", p=128))
        nc.sync.dma_start(dm12rot[:, RT:NT],
                          dm_out.ap()[bass.ds(ooff, r)].rearrange("(g p) -> p g", p=128))

    dve_ind = []   # deferred (col, lo) ind8 slices for the DVE tail

    with tc.tile_pool(name="p1c", bufs=2) as p1c, \
         tc.tile_pool(name="p1psA", bufs=3, space="PSUM") as psA, \
         tc.tile_pool(name="p1psT", bufs=3, space="PSUM") as psAT, \
         tc.tile_pool(name="p1psq", bufs=2, space="PSUM") as psq:
        for t in range(RT):
            # --- row-side A strips (fp8) + per-512-chunk max8 from PSUM ---
            cand = p1c.tile([128, NQ * 8], F32, tag="cand")
            top32 = p1c.tile([128, 32], F32, tag="top32")
            for s in range(NQ):
                ps = psA.tile([128, 512], F32, tag="ps")
                nc.tensor.matmul(ps[:], xn8ROT[:, t, :],
                                 xn8ROT[:, 4 * s:4 * s + 4, :])
                nc.vector.max(cand[:, s * 8:(s + 1) * 8], ps[:])
            # --- top-32 of candidates ---
            nc.vector.max(top32[:, 0:8], cand[:])
            for rnd in range(1, 4):
                nc.vector.match_replace(cand[:], top32[:, (rnd - 1) * 8:rnd * 8],
                                        cand[:], NEG_FILL)
                nc.vector.max(top32[:, rnd * 8:(rnd + 1) * 8], cand[:])
            # threshold (clamped at 0: raw-A masking == relu-A masking)
            nc.vector.tensor_scalar_max(thrM[:, t:t + 1], top32[:, 31:32], 0.0)
            # degree = sum(relu(top32)); max(max(v,0),v) == relu(v)
            dsc = p1c.tile([128, 32], F32, tag="dsc")
            nc.vector.scalar_tensor_tensor(dsc[:], top32[:], 0.0, top32[:],
                                           op0=Alu.max, op1=Alu.max,
                                           accum_out=degM[:, t:t + 1])
            # negthr[0, t*128:(t+1)*128] = -thr: PE transpose + ACT scale=-1
            # (no DRAM roundtrip: keeps the AT build ~1us behind the top-k)
            pthr = psq.tile([1, 128], F16, tag="pthr")
            nc.tensor.transpose(pthr[:], thrM[:, t:t + 1], id16[:])
            nc.scalar.activation(negthr[0:1, t * 128:(t + 1) * 128], pthr[:],
                                 Act.Copy, scale=-1.0)
            if t % 4 != 3:
                continue
            # --- AT build for this row quarter: A^T - thr, relu'd to fp8 ---
            qi = t // 4
            lo = qi * QW
            if t == RT - 1:
                # stage positive thresholds for the pass-2 broadcasts
                nc.sync.dma_start(
                    thr_dram.ap().rearrange("(t p) -> p t", p=128), thrM[:])
                emit_dm_chain()
            on_pool = qi < cfg.pool_quarters
            for c in range(NT):
                pat = psAT.tile([128, QW], F32, tag="pat")
                nc.tensor.matmul(pat[:], xn8ROT[:, c, :],
                                 xn8ROT[:, 4 * qi:4 * qi + 4, :],
                                 start=True, stop=False)
                nc.tensor.matmul(pat[:], ones1[:], negthr[0:1, lo:lo + QW],
                                 start=False, stop=True)
                nc.scalar.activation(MT_S[:, c, lo:lo + QW], pat[:], Act.Relu)
                if on_pool:
                    nc.gpsimd.tensor_scalar(IND[:, c, lo:lo + QW],
                                            MT_S[:, c, lo:lo + QW], 0.0, None,
                                            op0=Alu.is_gt)
                else:
                    dve_ind.append((c, lo))

    early_stack.close()  # xn8ROT/negthr dead after pass 1

    # =====================================================================
    # Pass 2/3: Chebyshev combines against MT_S/IND (fp8 DoubleRow)
    # =====================================================================
    late = stack.enter_context(tc.tile_pool(name="late", bufs=1))
    THRb = late.tile([128, r], F16, tag="THRb")     # becomes CT1b in place
    CDM1b = late.tile([128, r], F16, tag="CDM1b")
    Q1b = late.tile([128, r], F16, tag="Q1b")       # becomes Qh in place
    Q2b = late.tile([128, r], F16, tag="Q2b")       # becomes QtaT in place
    CDM2b = late.tile([128, r], F16, tag="CDM2b")
    CT2b = late.tile([128, r], F16, tag="CT2b")
    hTa = late.tile([128, r], F16, tag="hTa")
    T1T = late.tile([128, r], F16, tag="T1T")
    T2T = late.tile([128, r], F16, tag="T2T")
    T1loc = late.tile([128, RT, HDIM], F16, tag="T1loc")
    G8 = late.tile([128, NT, HDIM], F8, tag="G8")
    G28 = late.tile([128, NT, HDIM], F8, tag="G28")
    xres16 = late.tile([128, RT, DDIM], F16, tag="xres16")

    if cfg.b2:
        B2R = late.tile([128, ODIM], F32, tag="B2R")
        nc.sync.dma_start(B2R[:], b2_e.ap().partition_broadcast(128))

    def combine_pass(G, MT, which, psC, p2s):
        """Shifted Chebyshev product: psS = S^T x G, psI = ind^T x G per
        512-row strip (fp8 DoubleRow over j-tile pairs), then flat [h,row]
        assembly with the per-row coefficient broadcasts."""
        if which == "T1":
            CAb, CIb, ADDb = CDM1b, THRb, Q1b       # THRb==CT1b, Q1b==Qh here
        else:
            CAb, CIb, ADDb = CDM2b, CT2b, Q2b       # Q2b==QtaT here
        TT = T1T if which == "T1" else T2T
        for w in range(2):
            pss = []
            for rg in (2 * w, 2 * w + 1):
                psS = psC.tile([128, 512], F32, tag=f"psS{rg % 2}", name=f"psS{rg}")
                psI = psC.tile([128, 512], F32, tag=f"psI{rg % 2}", name=f"psI{rg}")
                pss.append((rg, psS, psI))
            for jp in range(NT // 2):
                st, sp = jp == 0, jp == NT // 2 - 1
                for rg, psS, psI in pss:
                    rsl = slice(rg * 512, (rg + 1) * 512)
                    nc.tensor.matmul(psS[:], G[:, 2 * jp:2 * jp + 2, :],
                                     MT[0][:, 2 * jp:2 * jp + 2, rsl],
                                     start=st, stop=sp, perf_mode=DRow)
                    nc.tensor.matmul(psI[:], G[:, 2 * jp:2 * jp + 2, :],
                                     MT[1][:, 2 * jp:2 * jp + 2, rsl],
                                     start=st, stop=sp, perf_mode=DRow)
            for rg, psS, psI in pss:
                rsl = slice(rg * 512, (rg + 1) * 512)
                u = p2s.tile([128, 512], F16, tag="u")
                v = p2s.tile([128, 512], F16, tag="v")
                nc.vector.tensor_tensor(u[:], psS[:], CAb[:, rsl], Alu.mult)
                nc.vector.tensor_tensor(v[:], psI[:], CIb[:, rsl], Alu.mult)
                nc.vector.tensor_tensor(u[:], u[:], v[:], Alu.add)
                nc.vector.tensor_tensor(TT[:, rsl], u[:], ADDb[:, rsl], Alu.add)

    # ---- pass 2: T1 ----
    t1iv = t1_in.ap().rearrange("(t p) d -> p t d", p=128)
    with tc.tile_pool(name="p2s", bufs=4) as p2s, \
         tc.tile_pool(name="p2ps", bufs=1, space="PSUM") as p2ps, \
         tc.tile_pool(name="p2psT", bufs=3, space="PSUM") as p2psT:
        # fp16 stagings of the per-row coefficient vectors -> broadcasts
        qstage = p2s.tile([128, 4, RT], F16, tag="qstage")
        for i, (row, dram) in enumerate([(CDM1, cdm1_dram), (Q1, q1_dram),
                                         (Q2, q2_dram), (CDM2, cdm2_dram)]):
            nc.scalar.copy(qstage[:, i, :], dmv[:, row, :])
            nc.sync.dma_start(dram.ap().rearrange("(t p) -> p t", p=128),
                              qstage[:, i, :])
        nc.sync.dma_start(CDM1b[:], cdm1_dram.ap().partition_broadcast(128))
        nc.sync.dma_start(Q1b[:], q1_dram.ap().partition_broadcast(128))
        nc.sync.dma_start(Q2b[:], q2_dram.ap().partition_broadcast(128))
        nc.sync.dma_start(CDM2b[:], cdm2_dram.ap().partition_broadcast(128))
        nc.sync.dma_start(THRb[:], thr_dram.ap().partition_broadcast(128))

        # deferred ind8 slices on DVE (fills the deg-exchange gap)
        for c, lo in dve_ind:
            nc.vector.tensor_scalar(IND[:, c, lo:lo + QW],
                                    MT_S[:, c, lo:lo + QW], 0.0, None,
                                    op0=Alu.is_gt)

        # G = dm12 * h in fp8 (2x_2p tensor_scalar)
        for g in range(NT):
            nc.vector.tensor_scalar_mul(G8[:, g, :], hROT[:, g, :],
                                        dm12rot[:, g:g + 1])
        # h^T strips for the y-stage and the Qh/QtaT combines
        for t in range(RT):
            phx = p2psT.tile([128, HDIM], F16, tag="ptrC")
            nc.tensor.transpose(phx[:], hROT[:, t, :], id16[:])
            nc.scalar.copy(hTa[:, t * 128:(t + 1) * 128], phx[:])
        # coefficient fixups (order matters: CT2b before THRb is overwritten)
        nc.vector.tensor_tensor(CT2b[:], CDM2b[:], THRb[:], Alu.mult)
        nc.vector.tensor_tensor(THRb[:], THRb[:], CDM1b[:], Alu.mult)  # CT1b
        nc.vector.tensor_tensor(Q1b[:], Q1b[:], hTa[:], Alu.mult)      # Qh

        combine_pass(G8, (MT_S, IND), "T1", p2ps, p2s)

        # T1 row-major (for G2 + exchange) via transposes
        for t in range(RT):
            pT1 = p2psT.tile([128, HDIM], F16, tag="ptrC")
            nc.tensor.transpose(pT1[:], T1T[:, t * 128:(t + 1) * 128], id16[:])
            nc.scalar.copy(T1loc[:, t, :], pT1[:])
        nc.sync.dma_start(t1iv[:], T1loc[:])
        nc.gpsimd.dma_start(xres16[:], xm.ap().rearrange("(t p) d -> p t d", p=128))
        # QtaT = q2*T1T - hTa, prebuilt while the T1 exchange is in flight
        nc.vector.tensor_tensor(Q2b[:], Q2b[:], T1T[:], Alu.mult)
        nc.vector.tensor_tensor(Q2b[:], Q2b[:], hTa[:], Alu.subtract)

    # T1 exchange
    if cfg.use_cc:
        nc.gpsimd.collective_compute("AllGather", Alu.bypass, replica_groups=groups,
                                     ins=[t1_in[:].opt()], outs=[t1_out[:].opt()])
    else:
        nc.sync.dma_start(t1_out[0:r, :], t1_in[:])
        if n > r:
            nc.sync.dma_start(t1_out[r:n, :], t1_in[:])

    # ---- pass 3: T2 combine ----
    with tc.tile_pool(name="p3G", bufs=1) as p3G, \
         tc.tile_pool(name="p3s", bufs=4) as p3s, \
         tc.tile_pool(name="p3ps", bufs=1, space="PSUM") as p3ps:
        # my half of G2 comes straight from local T1loc (no exchange dep):
        # the first strips of the T2 matmuls run during the exchange.
        for l in range(RT):
            nc.vector.tensor_scalar_mul(G28[:, l, :], T1loc[:, l, :],
                                        dm12rot[:, l:l + 1])
        T1oth = p3G.tile([128, RT, HDIM], F16, tag="T1oth")
        t1ovr = t1_out.ap()[bass.ds(ooff, r), :].rearrange("(g p) d -> p g d", p=128)
        for g0 in range(0, RT, 4):
            nc.sync.dma_start(T1oth[:, g0:g0 + 4, :], t1ovr[:, g0:g0 + 4, :])
            for g in range(g0, g0 + 4):
                nc.vector.tensor_scalar_mul(G28[:, RT + g, :], T1oth[:, g, :],
                                            dm12rot[:, RT + g:RT + g + 1])
        combine_pass(G28, (MT_S, IND), "T2", p3ps, p3s)

    # ---- output stage: y = [h,T1,T2] @ w2 (+b2); out = x + tanh(gate)*y ----
    with tc.tile_pool(name="po", bufs=1) as po, \
         tc.tile_pool(name="popsY", bufs=3, space="PSUM") as popsY:
        outt_all = po.tile([128, RT, ODIM], F32, tag="outt_all")
        oev = out_e.ap().rearrange("(t p) d -> p t d", p=128)
        # quad-wide: 12 matmuls into one psum tile, ONE 4-wide combine TSP
        for t0 in range(0, RT, 4):
            py4 = popsY.tile([128, 4, ODIM], F32, tag="pyY4")
            for ti in range(4):
                t = t0 + ti
                sl = slice(t * 128, (t + 1) * 128)
                comps = [hTa[:, sl], T1T[:, sl], T2T[:, sl]]
                for k in range(KCHEB):
                    nc.tensor.matmul(py4[:, ti, :], comps[k][:], w2s[:, k, :],
                                     start=(k == 0), stop=(k == KCHEB - 1))
                if cfg.b2:
                    nc.vector.tensor_tensor(py4[:, ti, :], py4[:, ti, :],
                                            B2R[:], Alu.add)
            nc.vector.scalar_tensor_tensor(outt_all[:, t0:t0 + 4, :], py4[:], tg,
                                           xres16[:, t0:t0 + 4, :],
                                           op0=Alu.mult, op1=Alu.add)
            nc.sync.dma_start(oev[:, t0:t0 + 4, :], outt_all[:, t0:t0 + 4, :])

    stack.close()


def build(cfg, num_devices):
    nc = bacc.Bacc("TRN2", target_bir_lowering=False, debug=False,
                   num_devices=num_devices)
    with tile.TileContext(nc) as tc:
        _emit(nc, tc, cfg)
    nc.compile()
    return nc


def _host_scalars(log_tau, gate):
    tau = max(float(np.exp(np.float32(log_tau))), 1e-3)
    c1 = (1.0 - TELEPORT) / tau
    c2 = (1.0 - TELEPORT) / tau + TELEPORT
    tg = float(np.tanh(np.float32(gate)))
    return c1, c2, tg


def _flags(ln_g, ln_b, b1, b2):
    return (not np.all(ln_g == 1.0), not np.all(ln_b == 0.0),
            not np.all(b1 == 0.0), not np.all(b2 == 0.0))


_CACHE = {}


def kernel(x, ln_g, ln_b, w1, b1, w2, b2, log_tau, gate):
    x = np.ascontiguousarray(x, dtype=np.float32)
    assert x.shape == (BSZ, NFULL, DDIM), x.shape
    scalars = _host_scalars(log_tau, gate)
    flags = _flags(np.asarray(ln_g), np.asarray(ln_b), np.asarray(b1), np.asarray(b2))
    key = (scalars, flags)
    if key not in _CACHE:
        cfg = Cfg(NFULL, NFULL // 2, True, scalars, flags)
        _CACHE[key] = (build(cfg, N_CORES), cfg)
    nc, cfg = _CACHE[key]

    r = cfg.r
    base = {
        "w1e": np.ascontiguousarray(w1, np.float32),
        "w2e": np.ascontiguousarray(w2, np.float32),
        "lng": np.ascontiguousarray(ln_g, np.float32),
        "lnb": np.ascontiguousarray(ln_b, np.float32),
        "b1e": np.ascontiguousarray(b1, np.float32),
        "b2e": np.ascontiguousarray(b2, np.float32),
    }
    in_maps = []
    for c in range(N_CORES):
        b, j = c // 2, c % 2
        m = dict(base)
        m["xf"] = x[b]
        m["xm"] = np.ascontiguousarray(x[b, j * r:(j + 1) * r, :])
        in_maps.append(m)

    res = run_bass_kernel_spmd(nc, in_maps, core_ids=list(range(N_CORES)))
    out = np.empty_like(x)
    for c in range(N_CORES):
        b, j = c // 2, c % 2
        out[b, j * r:(j + 1) * r, :] = res.results[c]["out"]
    return out
